# revision 42
# baseline (speedup 1.0000x reference)
"""Trainium2 Bass kernel for a transformer block with attention + top-2-of-4 MoE.

Problem (B=4, S=1024, D=768, H=12, E=4, DF=3072, TOPK=2):
  pooled task-routing bias -> pre-norm MHA with residual -> pre-norm top-2 MoE
  with routing bias, residual.

Sharding: 8 cores, token-parallel. Core i handles batch b=i//2, sequence half
i%2 (512 query tokens). Each core gets the full 1024-token sequence of its
batch (x_kv) to compute K/V and the pooled routing bias locally; no
collectives are needed. All weights are replicated, pre-cast to bf16 on the
host, with LayerNorm gammas/betas folded into the consuming weight matrices.

Attention runs in bf16 on the TensorEngine with fp32 PSUM accumulation;
LayerNorm statistics, softmax denominators and residuals stay fp32. Softmax
runs without max-subtraction (logits are O(1) for this problem's weight
scale), which lets scores be computed directly in [key, query] layout so no
attention-probability transposes are needed: the denominator comes from an
appended ones-column in the value tensor and 1/den = exp(-ln(den)) is applied
when evicting per-head context (vector.reciprocal on a [1, N] row is ~8x-slow
single-lane DVE work). The two heads of each pair occupy PE row groups 0 and
64 so their K=64 score matmuls run concurrently (a lone K=64 matmul streams
at half rate).

The MoE expert FFNs run as fp8e4m3 DoubleRow matmuls (weights host-quantized
with a x32 scale folded back out via the gelu input scale and a pre-scaled
gates vector), ~1.4x the bf16 TensorEngine throughput. Gate logits are
computed in full fp32 on the VectorEngine (overlapping expert matmuls) since
bf16 logit noise flips the top-2 selection on knife-edge tokens.
"""

import contextlib

import numpy as np
import ml_dtypes

import concourse.bass as bass
import concourse.bacc as bacc
import concourse.mybir as mybir
import concourse.tile as tile
from concourse import bass_isa, masks
from concourse.bass_utils import run_bass_kernel_spmd

B, S, D, H, E, DF = 4, 1024, 768, 12, 4, 3072
HD = D // H          # 64
NQ = S // 2          # 512 tokens owned per core
NKV = S
N_CORES = 8
DC = D // 128        # 6 d-chunks
FC = DF // 128       # 24 f-chunks
QT = NQ // 128       # 4 query token tiles
KT = NKV // 128      # 8 kv token tiles
EPS = 1e-5

F32 = mybir.dt.float32
BF16 = mybir.dt.bfloat16
AF = mybir.ActivationFunctionType
ALU = mybir.AluOpType
AX = mybir.AxisListType
BF16NP = ml_dtypes.bfloat16
FP8 = mybir.dt.float8e4
FP8NP = ml_dtypes.float8_e4m3
WSCALE = 32.0

_CACHED = {}


def _layernorm_tiles(nc, small, junkp, x_tiles, n_tiles, xn_pool, name,
                     eps_ap=None):
    """LN over [128, D] f32 tiles -> bf16 normalized tiles (gamma/beta folded
    into downstream weights on the host). Returns list of bf16 [128, D] tiles."""
    stats_s = small.tile([128, n_tiles], F32, tag=f"{name}_s", name=f"{name}_s")
    stats_q = small.tile([128, n_tiles], F32, tag=f"{name}_q", name=f"{name}_q")
    for t in range(n_tiles):
        nc.vector.reduce_sum(stats_s[:, t : t + 1], x_tiles[t][:], axis=AX.X)
        j = junkp.tile([128, D], BF16, tag="junk", bufs=2, name="junk")
        nc.scalar.activation(j[:], x_tiles[t][:], AF.Square,
                             accum_out=stats_q[:, t : t + 1])
    mu = small.tile([128, n_tiles], F32, tag=f"{name}_mu", name=f"{name}_mu")
    var = small.tile([128, n_tiles], F32, tag=f"{name}_var", name=f"{name}_var")
    nc.vector.tensor_scalar(out=mu[:], in0=stats_s[:], scalar1=1.0 / D,
                            scalar2=None, op0=ALU.mult)
    nc.vector.tensor_scalar(out=var[:], in0=stats_q[:], scalar1=1.0 / D,
                            scalar2=None, op0=ALU.mult)
    mu2 = small.tile([128, n_tiles], F32, tag=f"{name}_mu2", name=f"{name}_mu2")
    nc.vector.tensor_mul(mu2[:], mu[:], mu[:])
    nc.vector.tensor_sub(var[:], var[:], mu2[:])
    lnv = small.tile([128, n_tiles], F32, tag=f"{name}_lnv", name=f"{name}_lnv")
    nc.scalar.activation(lnv[:], var[:], AF.Ln, bias=eps_ap[:], scale=1.0)
    rstd = small.tile([128, n_tiles], F32, tag=f"{name}_rstd", name=f"{name}_rstd")
    nc.scalar.activation(rstd[:], lnv[:], AF.Exp, scale=-0.5)
    nmr = small.tile([128, n_tiles], F32, tag=f"{name}_nmr", name=f"{name}_nmr")
    nc.vector.tensor_mul(nmr[:], mu[:], rstd[:])
    nc.vector.tensor_scalar(out=nmr[:], in0=nmr[:], scalar1=-1.0, scalar2=None,
                            op0=ALU.mult)
    xn_tiles = []
    for t in range(n_tiles):
        xn = xn_pool.tile([128, D], BF16, tag=f"{name}_xn", bufs=n_tiles,
                          name=f"{name}_xn")
        nc.scalar.activation(xn[:], x_tiles[t][:], AF.Identity,
                             bias=nmr[:, t : t + 1], scale=rstd[:, t : t + 1])
        xn_tiles.append(xn)
    return xn_tiles, mu, rstd


def _transpose_to(nc, ps_tr, ident, xn_tiles, dst_pool, n_tok_tiles, name,
                  dst_bufs=6):
    """Transpose token-major bf16 tiles [128tok, D] into d-major tiles
    [128d, n_tok_tiles*128]. Returns list of DC tiles."""
    dst = []
    for d in range(DC):
        dt_ = dst_pool.tile([128, n_tok_tiles * 128], BF16, tag=f"{name}_T",
                            bufs=dst_bufs, name=f"{name}_T")
        ps = ps_tr.tile([128, 1024], BF16, tag="big", bufs=2, name="tr_ps")
        for t in range(n_tok_tiles):
            nc.tensor.transpose(ps[:, t * 128 : (t + 1) * 128],
                                xn_tiles[t][:, d * 128 : (d + 1) * 128], ident[:])
        nc.vector.tensor_copy(dt_[:], ps[:, : n_tok_tiles * 128])
        dst.append(dt_)
    return dst


def build_program(meta, stop_stage='full'):
    nc = bacc.Bacc("TRN2", target_bir_lowering=False, debug=False,
                   num_devices=N_CORES)

    # ---- DRAM parameters --------------------------------------------------
    xq_d = nc.declare_dram_parameter("x_q", [NQ, D], F32, isOutput=False)
    xkv_d = nc.declare_dram_parameter("x_kv", [NKV, D], F32, isOutput=False)
    wq_d = nc.declare_dram_parameter("wq", [D, D], BF16, isOutput=False)
    wk_d = nc.declare_dram_parameter("wk", [D, D], BF16, isOutput=False)
    wv_d = nc.declare_dram_parameter("wv", [D, D], BF16, isOutput=False)
    wo_d = nc.declare_dram_parameter("wo", [D, D], BF16, isOutput=False)
    wtt_d = nc.declare_dram_parameter("wt_t", [1, 2 * D], BF16, isOutput=False)
    wg_d = nc.declare_dram_parameter("wg_r", [128, DC * E], BF16, isOutput=False)
    bg_d = nc.declare_dram_parameter("bg", [1, E], F32, isOutput=False)
    wgb_d = nc.declare_dram_parameter("wg_bcast", [128, E * D], F32, isOutput=False)
    w1_d = nc.declare_dram_parameter("w1r", [E, DC, 128, 3 * 2 * 512], FP8, isOutput=False)
    w2_d = nc.declare_dram_parameter("w2r", [E, FC // 2, 128, 2 * D], FP8, isOutput=False)
    b1_d = nc.declare_dram_parameter("b1t", [128, E * FC], F32, isOutput=False)
    bqk_d = nc.declare_dram_parameter("bqk", [128, 2 * DC], F32, isOutput=False)
    out_d = nc.declare_dram_parameter("out", [NQ, D], F32, isOutput=True)

    has_qkbias = meta["has_qkbias"]
    bt0, bt1 = meta["bt0"], meta["bt1"]

    with tile.TileContext(nc) as tc, contextlib.ExitStack() as ctx:
        glob = ctx.enter_context(tc.tile_pool(name="glob", bufs=1))
        small = ctx.enter_context(tc.tile_pool(name="small", bufs=1))
        junkp = ctx.enter_context(tc.tile_pool(name="junkp", bufs=2))

        ps_tr = ctx.enter_context(tc.tile_pool(name="ps_tr", bufs=2, space="PSUM"))
        ps_mm = ctx.enter_context(tc.tile_pool(name="ps_mm", bufs=2, space="PSUM"))
        ps_cx = ctx.enter_context(tc.tile_pool(name="ps_cx", bufs=2, space="PSUM"))
        ps_s = ps_tr  # scores share the 2x [128,1024]-sized slots with transposes

        ident = glob.tile([128, 128], BF16, tag="ident", name="ident")
        masks.make_identity(nc, ident[:])
        eps_sb = glob.tile([128, 1], F32, tag="eps", name="eps_sb")
        nc.vector.memset(eps_sb[:], EPS)

        xq_sb = []
        for t in range(QT):
            xt = glob.tile([128, D], F32, tag="xq", bufs=QT, name="xq")
            nc.sync.dma_start(xt[:], xq_d[t * 128 : (t + 1) * 128, :])
            xq_sb.append(xt)

        # MoE weight streaming pools (w1 global so expert 0 prefetches early)
        w1p = ctx.enter_context(tc.tile_pool(name="w1p", bufs=2))
        b1t_sb = glob.tile([128, E * FC], F32, tag="b1t", name="b1t")
        nc.sync.dma_start(b1t_sb[:], b1_d[:])
        wg_sb = glob.tile([128, DC, E], BF16, tag="wg", name="wg")
        nc.sync.dma_start(wg_sb[:], wg_d[:].rearrange("p (d e) -> p d e", e=E))
        bg_sb = glob.tile([1, E], F32, tag="bg", name="bg")
        nc.sync.dma_start(bg_sb[:], bg_d[:])

        xres = [glob.tile([128, D], F32, tag="xres", bufs=QT, name="xres")
                for _ in range(QT)]
        gates = [glob.tile([128, E], F32, tag="gates", bufs=QT, name="gates")
                 for _ in range(QT)]
        bias4 = glob.tile([1, E], F32, tag="bias4", name="bias4")
        h2T = []

        # ---- attention phase ---------------------------------------------
        with (
            tc.tile_pool(name="attn", bufs=1) as attn,
            tc.tile_pool(name="xnp", bufs=8) as xnp,
        ):
            wsub_cm = tc.tile_pool(name="wsub", bufs=1)
            wsub = wsub_cm.__enter__()
            xkv_sb = []
            for t in range(KT):
                xt = attn.tile([128, D], F32, tag="xkv", bufs=KT, name="xkv")
                nc.sync.dma_start(xt[:], xkv_d[t * 128 : (t + 1) * 128, :])
                xkv_sb.append(xt)
            wq_sb, wk_sb, wv_sb, wo_sb = [], [], [], []
            for (wd, lst, tg) in ((wq_d, wq_sb, "wq"), (wk_d, wk_sb, "wk"),
                                  (wv_d, wv_sb, "wv")):
                for d in range(DC):
                    wt_ = wsub.tile([128, D], BF16, tag=tg, bufs=DC, name=tg)
                    nc.sync.dma_start(wt_[:], wd[d * 128 : (d + 1) * 128, :])
                    lst.append(wt_)
            for d in range(DC):
                wt2_ = attn.tile([128, D], BF16, tag="wo", bufs=DC, name="wo")
                nc.sync.dma_start(wt2_[:], wo_d[d * 128 : (d + 1) * 128, :])
                wo_sb.append(wt2_)
            wtt_sb = attn.tile([1, 2, D], BF16, tag="wtt", name="wtt")
            nc.sync.dma_start(wtt_sb[:], wtt_d[:].rearrange("p (j d) -> p j d", j=2))
            bqk_sb = attn.tile([128, 2 * DC], F32, tag="bqk", name="bqk")
            nc.sync.dma_start(bqk_sb[:], bqk_d[:])

            # pooled routing bias: pooled = mean_t x_kv; tl = pooled @ Wt + bt
            # (partition sum on GpSimd to keep it off the TensorEngine)
            pooled = attn.tile([1, D], F32, tag="pooled", name="pooled")
            psum_x = junkp.tile([128, D], BF16, tag="junk", bufs=2,
                                name="junk")
            nc.vector.tensor_add(psum_x[:], xkv_sb[0][:], xkv_sb[1][:])
            for t in range(2, KT):
                nc.vector.tensor_add(psum_x[:], psum_x[:], xkv_sb[t][:])
            pall = junkp.tile([128, D], BF16, tag="junk", bufs=2, name="junk")
            nc.gpsimd.partition_all_reduce(pall[:], psum_x[:], channels=128,
                                           reduce_op=bass_isa.ReduceOp.add)
            nc.vector.tensor_scalar(out=pooled[:], in0=pall[0:1, :],
                                    scalar1=1.0 / NKV, scalar2=None,
                                    op0=ALU.mult)
            tl = attn.tile([1, 2], F32, tag="tl", name="tl")
            for j, btj in ((0, bt0), (1, bt1)):
                jrow = junkp.tile([128, D], BF16, tag="junk", bufs=2, name="junk")
                nc.vector.tensor_mul(jrow[0:1, :], pooled[:], wtt_sb[:, j, :])
                nc.vector.reduce_sum(tl[:, j : j + 1], jrow[0:1, :], axis=AX.X)
                nc.vector.tensor_scalar(out=tl[:, j : j + 1],
                                        in0=tl[:, j : j + 1],
                                        scalar1=float(btj), scalar2=None,
                                        op0=ALU.add)
            for e_, j_ in ((0, 0), (1, 1), (2, 0), (3, 1)):  # DOMAIN_MAP
                nc.vector.tensor_copy(bias4[:, e_ : e_ + 1], tl[:, j_ : j_ + 1])
            nc.vector.tensor_add(bias4[:], bias4[:], bg_sb[:])
            bias4b = glob.tile([128, E], F32, tag="bias4b", name="bias4b")
            nc.gpsimd.partition_broadcast(bias4b[:], bias4[:])

            # LN1 -> transposed bf16 activations
            xn_kv, _, _ = _layernorm_tiles(nc, small, junkp, xkv_sb, KT, xnp, "ln1kv", eps_sb)
            xn_q, _, _ = _layernorm_tiles(nc, small, junkp, xq_sb, QT, xnp, "ln1q", eps_sb)
            hkvT = _transpose_to(nc, ps_tr, ident, xn_kv, attn, KT, "hkv")
            hqT = _transpose_to(nc, ps_tr, ident, xn_q, attn, QT, "hq")

            # Q/K projections -> d-major qT [D, NQ], kT [D, NKV]
            qT = [attn.tile([128, NQ], BF16, tag="qT", bufs=DC, name="qT")
                  for _ in range(DC)]
            kT = [attn.tile([128, NKV], BF16, tag="kT", bufs=DC, name="kT")
                  for _ in range(DC)]
            for m in range(DC):
                ps = ps_mm.tile([128, 512], F32, tag="mm", name="ps_q")
                for d in range(DC):
                    nc.tensor.matmul(ps[:], wq_sb[d][:, m * 128 : (m + 1) * 128],
                                     hqT[d][:], start=(d == 0), stop=(d == DC - 1))
                if has_qkbias:
                    nc.scalar.activation(qT[m][:], ps[:], AF.Identity,
                                         bias=bqk_sb[:, m : m + 1], scale=1.0)
                else:
                    nc.vector.tensor_copy(qT[m][:], ps[:])
                for hf in range(2):
                    ps2 = ps_mm.tile([128, 512], F32, tag="mm", name="ps_k")
                    for d in range(DC):
                        nc.tensor.matmul(
                            ps2[:], wk_sb[d][:, m * 128 : (m + 1) * 128],
                            hkvT[d][:, hf * 512 : (hf + 1) * 512],
                            start=(d == 0), stop=(d == DC - 1))
                    if has_qkbias:
                        nc.scalar.activation(kT[m][:, hf * 512 : (hf + 1) * 512],
                                             ps2[:], AF.Identity,
                                             bias=bqk_sb[:, DC + m : DC + m + 1],
                                             scale=1.0)
                    else:
                        nc.vector.tensor_copy(kT[m][:, hf * 512 : (hf + 1) * 512],
                                              ps2[:])

            # V projection (token-major) into v_aug [128, H, HD+1], ones col
            v_aug = [attn.tile([128, H, HD + 1], BF16, tag="vaug", bufs=KT,
                               name="vaug") for _ in range(KT)]
            for t in range(KT):
                for hf in range(2):
                    ps = ps_mm.tile([128, 512], F32, tag="mm", name="ps_v")
                    for d in range(DC):
                        nc.tensor.matmul(
                            ps[:, :384],
                            hkvT[d][:, t * 128 : (t + 1) * 128],
                            wv_sb[d][:, hf * 384 : (hf + 1) * 384],
                            start=(d == 0), stop=(d == DC - 1))
                    nc.vector.tensor_copy(
                        v_aug[t][:, hf * 6 : (hf + 1) * 6, :HD],
                        ps[:, :384].rearrange("p (h x) -> p h x", h=6))
                nc.vector.memset(v_aug[t][:, :, HD : HD + 1], 1.0)
            wsub_cm.__exit__(None, None, None)
            pp_cm = tc.tile_pool(name="pp", bufs=6)
            pp = pp_cm.__enter__()

            if stop_stage == "qkv":
                for t in range(QT):
                    nc.sync.dma_start(out_d[t * 128 : (t + 1) * 128, :],
                                      xq_sb[t][:])
            # head-pair attention: the two heads of a pair live in row
            # groups 0 and 64 of the same qT/kT tile, so their K=64 score
            # matmuls execute concurrently on disjoint PE row groups.
            ctxT = [attn.tile([128, NQ], BF16, tag="ctxT", bufs=DC, name="ctxT")
                    for _ in range(DC)]
            pairs = range(H // 2) if stop_stage != "qkv" else range(0)
            for j in pairs:
                p_sb = []
                for c in range(KT):
                    pss = ps_s.tile([128, 1024], F32, tag="big", bufs=2,
                                    name="ps_s")
                    nc.tensor.matmul(
                        pss[:, 0:512],
                        kT[j][0:HD, c * 128 : (c + 1) * 128],
                        qT[j][0:HD, :], start=True, stop=True)
                    nc.tensor.matmul(
                        pss[:, 512:1024],
                        kT[j][HD:128, c * 128 : (c + 1) * 128],
                        qT[j][HD:128, :], start=True, stop=True)
                    pc = pp.tile([128, 1024], BF16, tag="p", bufs=6, name="p")
                    nc.scalar.activation(pc[:], pss[:], AF.Exp,
                                         scale=float(1.0 / np.sqrt(HD)))
                    p_sb.append(pc)
                pcx0 = ps_cx.tile([HD + 1, 512], F32, tag="cx", name="ps_cx")
                pcx1 = ps_mm.tile([HD + 1, 512], F32, tag="mm", name="ps_cx1")
                for c in range(KT):
                    nc.tensor.matmul(pcx0[:], v_aug[c][:, 2 * j, :],
                                     p_sb[c][:, 0:512],
                                     start=(c == 0), stop=(c == KT - 1))
                    nc.tensor.matmul(pcx1[:], v_aug[c][:, 2 * j + 1, :],
                                     p_sb[c][:, 512:1024],
                                     start=(c == 0), stop=(c == KT - 1))
                # evict unnormalized ctx + denominators right away to free the
                # PSUM accumulators for the next pair; then normalize with
                # 1/den = exp(-ln(den)) computed on ScalarE (vector.reciprocal
                # on a single-partition row is ~8x-slow serial DVE work).
                den2 = attn.tile([1, 1024], BF16, tag="den2", bufs=1,
                                 name="den2")
                cu = attn.tile([64, 512], BF16, tag="cu", bufs=2, name="cu")
                nc.vector.tensor_copy(ctxT[j][0:HD, :], pcx0[:HD, :])
                nc.vector.tensor_copy(den2[:, 0:512], pcx0[HD : HD + 1, :])
                nc.vector.tensor_copy(cu[:], pcx1[:HD, :])
                nc.vector.tensor_copy(den2[:, 512:1024], pcx1[HD : HD + 1, :])
                lnd = attn.tile([1, 1024], F32, tag="lnd", bufs=1, name="lnd")
                nc.scalar.activation(lnd[:], den2[:], AF.Ln)
                rdenb2 = attn.tile([1, 1024], BF16, tag="rdenb2", bufs=1,
                                   name="rdenb2")
                nc.scalar.activation(rdenb2[:], lnd[:], AF.Exp, scale=-1.0)
                bcs0 = attn.tile([64, 512], BF16, tag="bcs", bufs=2,
                                 name="bcs")
                nc.gpsimd.partition_broadcast(bcs0[:], rdenb2[:, 0:512])
                nc.vector.tensor_mul(ctxT[j][0:HD, :], ctxT[j][0:HD, :],
                                     bcs0[:])
                bcs1 = attn.tile([64, 512], BF16, tag="bcs", bufs=2,
                                 name="bcs")
                nc.gpsimd.partition_broadcast(bcs1[:], rdenb2[:, 512:1024])
                nc.vector.tensor_mul(ctxT[j][HD:128, :], cu[:], bcs1[:])

            # output projection + residual -> xres (f32)
            for t in (range(QT) if stop_stage not in ("qkv", "heads") else range(0)):
                for hf in range(2):
                    ps = ps_mm.tile([128, 512], F32, tag="mm", name="ps_o")
                    for d in range(DC):
                        nc.tensor.matmul(
                            ps[:, :384],
                            ctxT[d][:, t * 128 : (t + 1) * 128],
                            wo_sb[d][:, hf * 384 : (hf + 1) * 384],
                            start=(d == 0), stop=(d == DC - 1))
                    nc.vector.tensor_add(xres[t][:, hf * 384 : (hf + 1) * 384],
                                         ps[:, :384],
                                         xq_sb[t][:, hf * 384 : (hf + 1) * 384])

            if stop_stage == "heads":
                for t in range(QT):
                    nc.sync.dma_start(out_d[t * 128 : (t + 1) * 128, :],
                                      xq_sb[t][:])
            if stop_stage == "attn":
                for t in range(QT):
                    nc.sync.dma_start(out_d[t * 128 : (t + 1) * 128, :],
                                      xres[t][:])
            # LN2 + transpose into h2T (kept in glob for MoE phase)
            do_rest = stop_stage in ("full", "gates")
            if do_rest:
                xn2, ln2mu, ln2rstd = _layernorm_tiles(
                    nc, small, junkp, xres, QT, xnp, "ln2", eps_sb)
                h2f8 = glob.tile([128, DC, 512], FP8, tag="h2f8", name="h2f8")
                for d in range(DC):
                    dt_ = glob.tile([128, 512], BF16, tag="h2_T", bufs=DC,
                                    name="h2_T")
                    ps = ps_tr.tile([128, 1024], BF16, tag="big", bufs=2,
                                    name="tr_ps")
                    for t in range(QT):
                        nc.tensor.transpose(ps[:, t * 128 : (t + 1) * 128],
                                            xn2[t][:, d * 128 : (d + 1) * 128],
                                            ident[:])
                    nc.vector.tensor_copy(dt_[:], ps[:, :512])
                    nc.scalar.copy(h2f8[:, d, :], ps[:, :512])
                    h2T.append(dt_)

            # gate logits + top-2 softmax gates
            pp_cm.__exit__(None, None, None)

        if stop_stage == "gates":
            for t in range(QT):
                nc.sync.dma_start(out_d[t * 128 : (t + 1) * 128, :], xres[t][:])
        # ---- MoE phase (fp8 DoubleRow matmuls) -----------------------------
        # W1/W2 are host-quantized to fp8e4 scaled by WSCALE; the 1/WSCALE is
        # folded into the gelu input scale (W1) and a pre-scaled gates vector
        # (W2). Contraction runs in K=256 DoubleRow chunks: both operands are
        # [128, 2, N] with the pair index selecting the two K-subchunks.
        with (
            tc.tile_pool(name="hidp", bufs=14) as hidp,
            tc.tile_pool(name="w2p", bufs=13) as w2p,
        ):
            gtp_cm = tc.tile_pool(name="gtp", bufs=1)
            gtp = gtp_cm.__enter__()
            gates32 = [glob.tile([128, E], F32, tag="gates32", bufs=QT,
                                 name="gates32") for _ in range(QT)]
            # Gate logits in full f32 on DVE (bf16 matmul noise flips the
            # top-2 selection on knife-edge tokens):
            #   logit[t, e] = rstd_t * (xres_t . wg_e - mu_t * sum(wg_e)) + bias_e
            if do_rest:
                wgb = [gtp.tile([128, D], F32, tag="wgb", bufs=E, name="wgb")
                       for _ in range(E)]
                for e_ in range(E):
                    nc.sync.dma_start(wgb[e_][:],
                                      wgb_d[:, e_ * D : (e_ + 1) * D])
            for t in (range(QT) if do_rest else range(0)):
                glog = gtp.tile([128, E], F32, tag="glog", bufs=2, name="glog")
                for e_ in range(E):
                    jr = gtp.tile([128, D], F32, tag="jr32", bufs=2,
                                 name="jr32")
                    nc.vector.tensor_mul(jr[:], xres[t][:], wgb[e_][:])
                    dot = gtp.tile([128, 1], F32, tag="gdot", bufs=2,
                                    name="gdot")
                    nc.vector.reduce_sum(dot[:], jr[:], axis=AX.X)
                    mterm = gtp.tile([128, 1], F32, tag="gmt", bufs=2,
                                      name="gmt")
                    nc.vector.tensor_scalar(
                        out=mterm[:], in0=ln2mu[:, t : t + 1],
                        scalar1=float(meta["swg"][e_]), scalar2=None,
                        op0=ALU.mult)
                    nc.vector.tensor_sub(dot[:], dot[:], mterm[:])
                    nc.vector.tensor_mul(dot[:], dot[:],
                                         ln2rstd[:, t : t + 1])
                    nc.vector.tensor_add(glog[:, e_ : e_ + 1], dot[:],
                                         bias4b[:, e_ : e_ + 1])
                m1 = gtp.tile([128, 1], F32, tag="m1", bufs=2, name="m1")
                nc.vector.reduce_max(m1[:], glog[:], axis=AX.X)
                eq1 = gtp.tile([128, E], F32, tag="eq1", bufs=2, name="eq1")
                nc.vector.tensor_scalar(out=eq1[:], in0=glog[:], scalar1=m1[:],
                                        scalar2=None, op0=ALU.is_equal)
                big = gtp.tile([128, E], F32, tag="big", bufs=2, name="big")
                nc.vector.tensor_scalar(out=big[:], in0=eq1[:], scalar1=1e30,
                                        scalar2=None, op0=ALU.mult)
                msk = gtp.tile([128, E], F32, tag="msk", bufs=2, name="msk")
                nc.vector.tensor_sub(msk[:], glog[:], big[:])
                m2 = gtp.tile([128, 1], F32, tag="m2", bufs=2, name="m2")
                nc.vector.reduce_max(m2[:], msk[:], axis=AX.X)
                eq2 = gtp.tile([128, E], F32, tag="eq2", bufs=2, name="eq2")
                nc.vector.tensor_scalar(out=eq2[:], in0=msk[:], scalar1=m2[:],
                                        scalar2=None, op0=ALU.is_equal)
                nm1 = gtp.tile([128, 1], F32, tag="nm1", bufs=2, name="nm1")
                nc.vector.tensor_scalar(out=nm1[:], in0=m1[:], scalar1=-1.0,
                                        scalar2=None, op0=ALU.mult)
                dx = gtp.tile([128, 1], F32, tag="dx", bufs=2, name="dx")
                nc.scalar.activation(dx[:], m2[:], AF.Exp, bias=nm1[:], scale=1.0)
                sden = gtp.tile([128, 1], F32, tag="sden", bufs=2, name="sden")
                nc.vector.tensor_scalar(out=sden[:], in0=dx[:], scalar1=1.0,
                                        scalar2=None, op0=ALU.add)
                w1s = gtp.tile([128, 1], F32, tag="w1s", bufs=2, name="w1s")
                nc.vector.reciprocal(w1s[:], sden[:])
                w2s = gtp.tile([128, 1], F32, tag="w2s", bufs=2, name="w2s")
                nc.vector.tensor_mul(w2s[:], dx[:], w1s[:])
                ga = gtp.tile([128, E], F32, tag="ga", bufs=2, name="ga")
                nc.vector.tensor_scalar(out=ga[:], in0=eq1[:], scalar1=w1s[:],
                                        scalar2=None, op0=ALU.mult)
                gb = gtp.tile([128, E], F32, tag="gb", bufs=2, name="gb")
                nc.vector.tensor_scalar(out=gb[:], in0=eq2[:], scalar1=w2s[:],
                                        scalar2=None, op0=ALU.mult)
                nc.vector.tensor_add(gates[t][:], ga[:], gb[:])
                nc.vector.tensor_scalar(out=gates32[t][:], in0=gates[t][:],
                                        scalar1=1.0 / WSCALE, scalar2=None,
                                        op0=ALU.mult)
            for e in (range(E) if stop_stage == "full" else range(0)):
                w2_sb = []
                for fp in range(FC // 2):
                    wt_ = w2p.tile([128, 2, D], FP8, tag="w2f", bufs=13,
                                   name="w2f")
                    nc.sync.dma_start(wt_[:], w2_d[e, fp].rearrange(
                        "p (i d) -> p i d", i=2))
                    w2_sb.append(wt_)

                hidT = []
                for fcg in range(DC):
                    w1g = w1p.tile([128, 3, 2, 512], FP8, tag="w1g", bufs=2,
                                   name="w1g")
                    nc.sync.dma_start(w1g[:], w1_d[e, fcg].rearrange(
                        "p (c i f) -> p c i f", c=3, i=2))
                    for fcm in range(4):
                        fc = fcg * 4 + fcm
                        ps = ps_cx.tile([128, 512], F32, tag="cx", name="ps_h")
                        for c in range(3):
                            nc.tensor.matmul(
                                ps[:],
                                w1g[:, c, :, fcm * 128 : (fcm + 1) * 128],
                                h2f8[:, 2 * c : 2 * c + 2, :],
                                start=(c == 0), stop=(c == 2),
                                perf_mode=mybir.MatmulPerfMode.DoubleRow)
                        if fc % 2 == 0:
                            hpair = hidp.tile([128, 2, 512], FP8, tag="hid",
                                              bufs=14, name="hid")
                            hidT.append(hpair)
                        nc.scalar.activation(
                            hidT[fc // 2][:, fc % 2, :], ps[:],
                            AF.Gelu_apprx_tanh,
                            bias=b1t_sb[:, e * FC + fc : e * FC + fc + 1],
                            scale=1.0 / WSCALE)

                for t in range(QT):
                    pya = ps_mm.tile([128, 512], F32, tag="mm", name="ps_ya")
                    pyb = ps_mm.tile([128, 512], F32, tag="mm", name="ps_yb")
                    for fp in range(FC // 2):
                        nc.tensor.matmul(
                            pya[:, :384],
                            hidT[fp][:, :, t * 128 : (t + 1) * 128],
                            w2_sb[fp][:, :, 0:384],
                            start=(fp == 0), stop=(fp == FC // 2 - 1),
                            perf_mode=mybir.MatmulPerfMode.DoubleRow)
                        nc.tensor.matmul(
                            pyb[:, :384],
                            hidT[fp][:, :, t * 128 : (t + 1) * 128],
                            w2_sb[fp][:, :, 384:768],
                            start=(fp == 0), stop=(fp == FC // 2 - 1),
                            perf_mode=mybir.MatmulPerfMode.DoubleRow)
                    for hf, py in ((0, pya), (1, pyb)):
                        ys = junkp.tile([128, 384], F32, tag="ys", bufs=2,
                                        name="ys")
                        nc.scalar.activation(ys[:], py[:, :384], AF.Identity,
                                             bias=0.0,
                                             scale=gates32[t][:, e : e + 1])
                        nc.vector.tensor_add(
                            xres[t][:, hf * 384 : (hf + 1) * 384],
                            xres[t][:, hf * 384 : (hf + 1) * 384], ys[:])
                    if e == E - 1:
                        nc.sync.dma_start(out_d[t * 128 : (t + 1) * 128, :],
                                          xres[t][:])
            gtp_cm.__exit__(None, None, None)

    nc.compile()
    return nc


def prepare_inputs(inputs):
    x = np.asarray(inputs["x"], np.float32)
    ln1_g = np.asarray(inputs["ln1_g"], np.float32)
    ln1_b = np.asarray(inputs["ln1_b"], np.float32)
    Wq = np.asarray(inputs["Wq"], np.float32)
    Wk = np.asarray(inputs["Wk"], np.float32)
    Wv = np.asarray(inputs["Wv"], np.float32)
    Wo = np.asarray(inputs["Wo"], np.float32)
    Wt = np.asarray(inputs["Wt"], np.float32)
    bt = np.asarray(inputs["bt"], np.float32)
    ln2_g = np.asarray(inputs["ln2_g"], np.float32)
    ln2_b = np.asarray(inputs["ln2_b"], np.float32)
    Wg = np.asarray(inputs["Wg"], np.float32)
    W1 = np.asarray(inputs["W1"], np.float32)
    b1 = np.asarray(inputs["b1"], np.float32)
    W2 = np.asarray(inputs["W2"], np.float32)
    b2 = np.asarray(inputs["b2"], np.float32)

    bv = ln1_b @ Wv
    if np.any(b2) or np.any(bv):
        raise NotImplementedError("nonzero b2 / ln1_b@Wv path not implemented")

    wq = (ln1_g[:, None] * Wq).astype(BF16NP)
    wk = (ln1_g[:, None] * Wk).astype(BF16NP)
    wv = (ln1_g[:, None] * Wv).astype(BF16NP)
    wo = Wo.astype(BF16NP)
    bq = ln1_b @ Wq
    bk = ln1_b @ Wk
    bqk = np.concatenate([bq.reshape(DC, 128).T, bk.reshape(DC, 128).T],
                         axis=1).astype(np.float32)
    has_qkbias = bool(np.any(bqk))

    wg = (ln2_g[:, None] * Wg).astype(BF16NP)
    wg_r = np.ascontiguousarray(
        wg.reshape(DC, 128, E).transpose(1, 0, 2).reshape(128, DC * E))
    bg = (ln2_b @ Wg).reshape(1, E).astype(np.float32)
    wg32 = (ln2_g[:, None] * Wg).astype(np.float32)
    wg_bcast = np.ascontiguousarray(np.broadcast_to(
        wg32.T.reshape(1, E * D), (128, E * D)))
    swg = wg32.sum(axis=0)

    # fp8 DoubleRow layout for W1: [e, fcg, p, (c i fgrp)] where the
    # contraction index is d = c*256 + i*128 + p.
    w1 = ((ln2_g[None, :, None] * W1) * WSCALE).astype(FP8NP)
    w1r = np.ascontiguousarray(
        w1.reshape(E, 3, 2, 128, DC, 512).transpose(0, 4, 3, 1, 2, 5)
        .reshape(E, DC, 128, 3 * 2 * 512))
    b1_tot = (b1 + np.einsum("d,edf->ef", ln2_b, W1)).astype(np.float32)
    b1t = np.ascontiguousarray(
        b1_tot.reshape(E, FC, 128).transpose(2, 0, 1).reshape(128, E * FC))
    # fp8 DoubleRow layout for W2: [e, fp, p, (i d)], contraction index
    # f = fp*256 + i*128 + p.
    w2 = (W2 * WSCALE).astype(FP8NP)
    w2r = np.ascontiguousarray(
        w2.reshape(E, FC // 2, 2, 128, D).transpose(0, 1, 3, 2, 4)
        .reshape(E, FC // 2, 128, 2 * D))
    wt_t = np.ascontiguousarray(Wt.T).astype(BF16NP).reshape(1, 2 * D)

    meta = {"bt0": float(bt[0]), "bt1": float(bt[1]),
            "has_qkbias": has_qkbias, "swg": [float(v) for v in swg]}
    shared = {
        "wq": wq, "wk": wk, "wv": wv, "wo": wo, "wt_t": wt_t,
        "wg_r": wg_r, "bg": bg, "w1r": w1r, "w2r": w2r, "b1t": b1t,
        "bqk": bqk, "wg_bcast": wg_bcast,
    }
    in_maps = []
    for i in range(N_CORES):
        b, half = i // 2, i % 2
        m = dict(shared)
        m["x_kv"] = np.ascontiguousarray(x[b])
        m["x_q"] = np.ascontiguousarray(x[b, half * NQ : (half + 1) * NQ])
        in_maps.append(m)
    return meta, in_maps


def kernel(**inputs):
    meta, in_maps = prepare_inputs(inputs)
    key = ("v1", meta["has_qkbias"], meta["bt0"], meta["bt1"], tuple(meta["swg"]))
    if key not in _CACHED:
        _CACHED[key] = build_program(meta)
    nc = _CACHED[key]

    res = run_bass_kernel_spmd(nc, in_maps, list(range(N_CORES)),
                               trace=bool(inputs.get("_trace", False)))
    out = np.empty((B, S, D), np.float32)
    for i in range(N_CORES):
        b, half = i // 2, i % 2
        out[b, half * NQ : (half + 1) * NQ] = res.results[i]["out"]
    if inputs.get("_want_time", False):
        return out, res
    return out


# revision 43
# speedup vs baseline: 1.0387x; 1.0387x over previous
"""Trainium2 Bass kernel for a transformer block with attention + top-2-of-4 MoE.

Problem (B=4, S=1024, D=768, H=12, E=4, DF=3072, TOPK=2):
  pooled task-routing bias -> pre-norm MHA with residual -> pre-norm top-2 MoE
  with routing bias, residual.

Sharding: 8 cores, token-parallel. Core i handles batch b=i//2, sequence half
i%2 (512 query tokens). Each core gets the full 1024-token sequence of its
batch (x_kv) to compute K/V and the pooled routing bias locally; no
collectives are needed. All weights are replicated, pre-cast to bf16 on the
host, with LayerNorm gammas/betas folded into the consuming weight matrices.

Attention runs in bf16 on the TensorEngine with fp32 PSUM accumulation;
LayerNorm statistics, softmax denominators and residuals stay fp32. Softmax
runs without max-subtraction (logits are O(1) for this problem's weight
scale), which lets scores be computed directly in [key, query] layout so no
attention-probability transposes are needed: the denominator comes from an
appended ones-column in the value tensor and 1/den = exp(-ln(den)) is applied
when evicting per-head context (vector.reciprocal on a [1, N] row is ~8x-slow
single-lane DVE work). The two heads of each pair occupy PE row groups 0 and
64 so their K=64 score matmuls run concurrently (a lone K=64 matmul streams
at half rate).

The MoE expert FFNs run as fp8e4m3 DoubleRow matmuls (weights host-quantized
with a x32 scale folded back out via the gelu input scale and a pre-scaled
gates vector), ~1.4x the bf16 TensorEngine throughput. Gate logits are
computed in full fp32 on the VectorEngine (overlapping expert matmuls) since
bf16 logit noise flips the top-2 selection on knife-edge tokens.
"""

import contextlib

import numpy as np
import ml_dtypes

import concourse.bass as bass
import concourse.bacc as bacc
import concourse.mybir as mybir
import concourse.tile as tile
from concourse import bass_isa, masks
from concourse.bass_utils import run_bass_kernel_spmd

B, S, D, H, E, DF = 4, 1024, 768, 12, 4, 3072
HD = D // H          # 64
NQ = S // 2          # 512 tokens owned per core
NKV = S
N_CORES = 8
DC = D // 128        # 6 d-chunks
FC = DF // 128       # 24 f-chunks
QT = NQ // 128       # 4 query token tiles
KT = NKV // 128      # 8 kv token tiles
EPS = 1e-5

F32 = mybir.dt.float32
BF16 = mybir.dt.bfloat16
AF = mybir.ActivationFunctionType
ALU = mybir.AluOpType
AX = mybir.AxisListType
BF16NP = ml_dtypes.bfloat16
FP8 = mybir.dt.float8e4
FP8NP = ml_dtypes.float8_e4m3
WSCALE = 32.0

_CACHED = {}


def _layernorm_tiles(nc, small, junkp, x_tiles, n_tiles, xn_pool, name,
                     eps_ap=None):
    """LN over [128, D] f32 tiles -> bf16 normalized tiles (gamma/beta folded
    into downstream weights on the host). Returns list of bf16 [128, D] tiles."""
    stats_s = small.tile([128, n_tiles], F32, tag=f"{name}_s", name=f"{name}_s")
    stats_q = small.tile([128, n_tiles], F32, tag=f"{name}_q", name=f"{name}_q")
    for t in range(n_tiles):
        nc.vector.reduce_sum(stats_s[:, t : t + 1], x_tiles[t][:], axis=AX.X)
        j = junkp.tile([128, D], BF16, tag="junk", bufs=2, name="junk")
        nc.scalar.activation(j[:], x_tiles[t][:], AF.Square,
                             accum_out=stats_q[:, t : t + 1])
    mu = small.tile([128, n_tiles], F32, tag=f"{name}_mu", name=f"{name}_mu")
    var = small.tile([128, n_tiles], F32, tag=f"{name}_var", name=f"{name}_var")
    nc.vector.tensor_scalar(out=mu[:], in0=stats_s[:], scalar1=1.0 / D,
                            scalar2=None, op0=ALU.mult)
    nc.vector.tensor_scalar(out=var[:], in0=stats_q[:], scalar1=1.0 / D,
                            scalar2=None, op0=ALU.mult)
    mu2 = small.tile([128, n_tiles], F32, tag=f"{name}_mu2", name=f"{name}_mu2")
    nc.vector.tensor_mul(mu2[:], mu[:], mu[:])
    nc.vector.tensor_sub(var[:], var[:], mu2[:])
    lnv = small.tile([128, n_tiles], F32, tag=f"{name}_lnv", name=f"{name}_lnv")
    nc.scalar.activation(lnv[:], var[:], AF.Ln, bias=eps_ap[:], scale=1.0)
    rstd = small.tile([128, n_tiles], F32, tag=f"{name}_rstd", name=f"{name}_rstd")
    nc.scalar.activation(rstd[:], lnv[:], AF.Exp, scale=-0.5)
    nmr = small.tile([128, n_tiles], F32, tag=f"{name}_nmr", name=f"{name}_nmr")
    nc.vector.tensor_mul(nmr[:], mu[:], rstd[:])
    nc.vector.tensor_scalar(out=nmr[:], in0=nmr[:], scalar1=-1.0, scalar2=None,
                            op0=ALU.mult)
    xn_tiles = []
    for t in range(n_tiles):
        xn = xn_pool.tile([128, D], BF16, tag=f"{name}_xn", bufs=n_tiles,
                          name=f"{name}_xn")
        nc.scalar.activation(xn[:], x_tiles[t][:], AF.Identity,
                             bias=nmr[:, t : t + 1], scale=rstd[:, t : t + 1])
        xn_tiles.append(xn)
    return xn_tiles, mu, rstd


def _transpose_to(nc, ps_tr, ident, xn_tiles, dst_pool, n_tok_tiles, name,
                  dst_bufs=6):
    """Transpose token-major bf16 tiles [128tok, D] into d-major tiles
    [128d, n_tok_tiles*128]. Returns list of DC tiles."""
    dst = []
    for d in range(DC):
        dt_ = dst_pool.tile([128, n_tok_tiles * 128], BF16, tag=f"{name}_T",
                            bufs=dst_bufs, name=f"{name}_T")
        ps = ps_tr.tile([128, 1024], BF16, tag="big", bufs=2, name="tr_ps")
        for t in range(n_tok_tiles):
            nc.tensor.transpose(ps[:, t * 128 : (t + 1) * 128],
                                xn_tiles[t][:, d * 128 : (d + 1) * 128], ident[:])
        nc.vector.tensor_copy(dt_[:], ps[:, : n_tok_tiles * 128])
        dst.append(dt_)
    return dst


def build_program(meta, stop_stage='full'):
    nc = bacc.Bacc("TRN2", target_bir_lowering=False, debug=False,
                   num_devices=N_CORES)

    # ---- DRAM parameters --------------------------------------------------
    xq_d = nc.declare_dram_parameter("x_q", [NQ, D], F32, isOutput=False)
    xkv_d = nc.declare_dram_parameter("x_kv", [NKV, D], F32, isOutput=False)
    wq_d = nc.declare_dram_parameter("wq", [D, D], BF16, isOutput=False)
    wk_d = nc.declare_dram_parameter("wk", [D, D], BF16, isOutput=False)
    wv_d = nc.declare_dram_parameter("wv", [D, D], BF16, isOutput=False)
    wo_d = nc.declare_dram_parameter("wo", [D, D], BF16, isOutput=False)
    wtt_d = nc.declare_dram_parameter("wt_t", [1, 2 * D], BF16, isOutput=False)
    wg_d = nc.declare_dram_parameter("wg_r", [128, DC * E], BF16, isOutput=False)
    bg_d = nc.declare_dram_parameter("bg", [1, E], F32, isOutput=False)
    wgb_d = nc.declare_dram_parameter("wg_bcast", [128, E * D], F32, isOutput=False)
    w1_d = nc.declare_dram_parameter("w1r", [E, DC, 128, 3 * 2 * 512], FP8, isOutput=False)
    w2_d = nc.declare_dram_parameter("w2r", [E, FC // 2, 128, 2 * D], FP8, isOutput=False)
    b1_d = nc.declare_dram_parameter("b1t", [128, E * FC], F32, isOutput=False)
    bqk_d = nc.declare_dram_parameter("bqk", [128, 2 * DC], F32, isOutput=False)
    out_d = nc.declare_dram_parameter("out", [NQ, D], F32, isOutput=True)

    has_qkbias = meta["has_qkbias"]
    bt0, bt1 = meta["bt0"], meta["bt1"]

    with tile.TileContext(nc) as tc, contextlib.ExitStack() as ctx:
        glob = ctx.enter_context(tc.tile_pool(name="glob", bufs=1))
        small = ctx.enter_context(tc.tile_pool(name="small", bufs=1))
        junkp = ctx.enter_context(tc.tile_pool(name="junkp", bufs=2))

        ps_tr = ctx.enter_context(tc.tile_pool(name="ps_tr", bufs=2, space="PSUM"))
        ps_mm = ctx.enter_context(tc.tile_pool(name="ps_mm", bufs=2, space="PSUM"))
        ps_cx = ctx.enter_context(tc.tile_pool(name="ps_cx", bufs=2, space="PSUM"))
        ps_s = ps_tr  # scores share the 2x [128,1024]-sized slots with transposes

        ident = glob.tile([128, 128], BF16, tag="ident", name="ident")
        masks.make_identity(nc, ident[:])
        eps_sb = glob.tile([128, 1], F32, tag="eps", name="eps_sb")
        nc.vector.memset(eps_sb[:], EPS)

        xq_sb = []
        for t in range(QT):
            xt = glob.tile([128, D], F32, tag="xq", bufs=QT, name="xq")
            nc.sync.dma_start(xt[:], xq_d[t * 128 : (t + 1) * 128, :])
            xq_sb.append(xt)

        # MoE weight streaming pools (w1 global so expert 0 prefetches early)
        w1p = ctx.enter_context(tc.tile_pool(name="w1p", bufs=2))
        b1t_sb = glob.tile([128, E * FC], F32, tag="b1t", name="b1t")
        nc.sync.dma_start(b1t_sb[:], b1_d[:])
        wg_sb = glob.tile([128, DC, E], BF16, tag="wg", name="wg")
        nc.sync.dma_start(wg_sb[:], wg_d[:].rearrange("p (d e) -> p d e", e=E))
        bg_sb = glob.tile([1, E], F32, tag="bg", name="bg")
        nc.sync.dma_start(bg_sb[:], bg_d[:])

        xres = [glob.tile([128, D], F32, tag="xres", bufs=QT, name="xres")
                for _ in range(QT)]
        gates = [glob.tile([128, E], F32, tag="gates", bufs=QT, name="gates")
                 for _ in range(QT)]
        bias4 = glob.tile([1, E], F32, tag="bias4", name="bias4")
        h2T = []

        # ---- attention phase ---------------------------------------------
        with (
            tc.tile_pool(name="attn", bufs=1) as attn,
            tc.tile_pool(name="xnp", bufs=8) as xnp,
        ):
            wsub_cm = tc.tile_pool(name="wsub", bufs=1)
            wsub = wsub_cm.__enter__()
            xkv_sb = []
            for t in range(KT):
                xt = attn.tile([128, D], F32, tag="xkv", bufs=KT, name="xkv")
                nc.sync.dma_start(xt[:], xkv_d[t * 128 : (t + 1) * 128, :])
                xkv_sb.append(xt)
            wq_sb, wk_sb, wv_sb, wo_sb = [], [], [], []
            for (wd, lst, tg) in ((wq_d, wq_sb, "wq"), (wk_d, wk_sb, "wk"),
                                  (wv_d, wv_sb, "wv")):
                for d in range(DC):
                    wt_ = wsub.tile([128, D], BF16, tag=tg, bufs=DC, name=tg)
                    nc.sync.dma_start(wt_[:], wd[d * 128 : (d + 1) * 128, :])
                    lst.append(wt_)
            for d in range(DC):
                wt2_ = attn.tile([128, D], BF16, tag="wo", bufs=DC, name="wo")
                nc.sync.dma_start(wt2_[:], wo_d[d * 128 : (d + 1) * 128, :])
                wo_sb.append(wt2_)
            wtt_sb = attn.tile([1, 2, D], BF16, tag="wtt", name="wtt")
            nc.sync.dma_start(wtt_sb[:], wtt_d[:].rearrange("p (j d) -> p j d", j=2))
            bqk_sb = attn.tile([128, 2 * DC], F32, tag="bqk", name="bqk")
            nc.sync.dma_start(bqk_sb[:], bqk_d[:])

            # pooled routing bias: pooled = mean_t x_kv; tl = pooled @ Wt + bt
            # (partition sum on GpSimd to keep it off the TensorEngine)
            pooled = attn.tile([1, D], F32, tag="pooled", name="pooled")
            psum_x = junkp.tile([128, D], BF16, tag="junk", bufs=2,
                                name="junk")
            nc.vector.tensor_add(psum_x[:], xkv_sb[0][:], xkv_sb[1][:])
            for t in range(2, KT):
                nc.vector.tensor_add(psum_x[:], psum_x[:], xkv_sb[t][:])
            pall = junkp.tile([128, D], BF16, tag="junk", bufs=2, name="junk")
            nc.gpsimd.partition_all_reduce(pall[:], psum_x[:], channels=128,
                                           reduce_op=bass_isa.ReduceOp.add)
            nc.vector.tensor_scalar(out=pooled[:], in0=pall[0:1, :],
                                    scalar1=1.0 / NKV, scalar2=None,
                                    op0=ALU.mult)
            tl = attn.tile([1, 2], F32, tag="tl", name="tl")
            for j, btj in ((0, bt0), (1, bt1)):
                jrow = junkp.tile([128, D], BF16, tag="junk", bufs=2, name="junk")
                nc.vector.tensor_mul(jrow[0:1, :], pooled[:], wtt_sb[:, j, :])
                nc.vector.reduce_sum(tl[:, j : j + 1], jrow[0:1, :], axis=AX.X)
                nc.vector.tensor_scalar(out=tl[:, j : j + 1],
                                        in0=tl[:, j : j + 1],
                                        scalar1=float(btj), scalar2=None,
                                        op0=ALU.add)
            for e_, j_ in ((0, 0), (1, 1), (2, 0), (3, 1)):  # DOMAIN_MAP
                nc.vector.tensor_copy(bias4[:, e_ : e_ + 1], tl[:, j_ : j_ + 1])
            nc.vector.tensor_add(bias4[:], bias4[:], bg_sb[:])
            bias4b = glob.tile([128, E], F32, tag="bias4b", name="bias4b")
            nc.gpsimd.partition_broadcast(bias4b[:], bias4[:])

            # LN1 -> transposed bf16 activations
            xn_kv, _, _ = _layernorm_tiles(nc, small, junkp, xkv_sb, KT, xnp, "ln1kv", eps_sb)
            xn_q, _, _ = _layernorm_tiles(nc, small, junkp, xq_sb, QT, xnp, "ln1q", eps_sb)
            hkvT = _transpose_to(nc, ps_tr, ident, xn_kv, attn, KT, "hkv")
            hqT = _transpose_to(nc, ps_tr, ident, xn_q, attn, QT, "hq")

            # Q/K projections -> d-major qT [D, NQ], kT [D, NKV]
            qT = [attn.tile([128, NQ], BF16, tag="qT", bufs=DC, name="qT")
                  for _ in range(DC)]
            kT = [attn.tile([128, NKV], BF16, tag="kT", bufs=DC, name="kT")
                  for _ in range(DC)]
            for m in range(DC):
                ps = ps_mm.tile([128, 512], F32, tag="mm", name="ps_q")
                for d in range(DC):
                    nc.tensor.matmul(ps[:], wq_sb[d][:, m * 128 : (m + 1) * 128],
                                     hqT[d][:], start=(d == 0), stop=(d == DC - 1))
                if has_qkbias:
                    nc.scalar.activation(qT[m][:], ps[:], AF.Identity,
                                         bias=bqk_sb[:, m : m + 1], scale=1.0)
                else:
                    nc.vector.tensor_copy(qT[m][:], ps[:])
                for hf in range(2):
                    ps2 = ps_mm.tile([128, 512], F32, tag="mm", name="ps_k")
                    for d in range(DC):
                        nc.tensor.matmul(
                            ps2[:], wk_sb[d][:, m * 128 : (m + 1) * 128],
                            hkvT[d][:, hf * 512 : (hf + 1) * 512],
                            start=(d == 0), stop=(d == DC - 1))
                    if has_qkbias:
                        nc.scalar.activation(kT[m][:, hf * 512 : (hf + 1) * 512],
                                             ps2[:], AF.Identity,
                                             bias=bqk_sb[:, DC + m : DC + m + 1],
                                             scale=1.0)
                    else:
                        nc.vector.tensor_copy(kT[m][:, hf * 512 : (hf + 1) * 512],
                                              ps2[:])

            # V projection (token-major) into v_aug [128, H, HD+1], ones col
            v_aug = [attn.tile([128, H, HD + 1], BF16, tag="vaug", bufs=KT,
                               name="vaug") for _ in range(KT)]
            for t in range(KT):
                for hf in range(2):
                    ps = ps_mm.tile([128, 512], F32, tag="mm", name="ps_v")
                    for d in range(DC):
                        nc.tensor.matmul(
                            ps[:, :384],
                            hkvT[d][:, t * 128 : (t + 1) * 128],
                            wv_sb[d][:, hf * 384 : (hf + 1) * 384],
                            start=(d == 0), stop=(d == DC - 1))
                    nc.vector.tensor_copy(
                        v_aug[t][:, hf * 6 : (hf + 1) * 6, :HD],
                        ps[:, :384].rearrange("p (h x) -> p h x", h=6))
                nc.vector.memset(v_aug[t][:, :, HD : HD + 1], 1.0)
            wsub_cm.__exit__(None, None, None)
            pp_cm = tc.tile_pool(name="pp", bufs=6)
            pp = pp_cm.__enter__()

            if stop_stage == "qkv":
                for t in range(QT):
                    nc.sync.dma_start(out_d[t * 128 : (t + 1) * 128, :],
                                      xq_sb[t][:])
            # head-pair attention: the two heads of a pair live in row
            # groups 0 and 64 of the same qT/kT tile, so their K=64 score
            # matmuls execute concurrently on disjoint PE row groups.
            ctxT = [attn.tile([128, NQ], BF16, tag="ctxT", bufs=DC, name="ctxT")
                    for _ in range(DC)]
            pairs = range(H // 2) if stop_stage != "qkv" else range(0)
            for j in pairs:
                p_sb = []
                for c in range(KT):
                    pss = ps_s.tile([128, 1024], F32, tag="big", bufs=2,
                                    name="ps_s")
                    nc.tensor.matmul(
                        pss[:, 0:512],
                        kT[j][0:HD, c * 128 : (c + 1) * 128],
                        qT[j][0:HD, :], start=True, stop=True)
                    nc.tensor.matmul(
                        pss[:, 512:1024],
                        kT[j][HD:128, c * 128 : (c + 1) * 128],
                        qT[j][HD:128, :], start=True, stop=True)
                    pc = pp.tile([128, 1024], BF16, tag="p", bufs=6, name="p")
                    nc.scalar.activation(pc[:], pss[:], AF.Exp,
                                         scale=float(1.0 / np.sqrt(HD)))
                    p_sb.append(pc)
                pcx0 = ps_cx.tile([HD + 1, 512], F32, tag="cx", name="ps_cx")
                pcx1 = ps_mm.tile([HD + 1, 512], F32, tag="mm", name="ps_cx1")
                for c in range(KT):
                    nc.tensor.matmul(pcx0[:], v_aug[c][:, 2 * j, :],
                                     p_sb[c][:, 0:512],
                                     start=(c == 0), stop=(c == KT - 1))
                    nc.tensor.matmul(pcx1[:], v_aug[c][:, 2 * j + 1, :],
                                     p_sb[c][:, 512:1024],
                                     start=(c == 0), stop=(c == KT - 1))
                # evict unnormalized ctx + denominators right away to free the
                # PSUM accumulators for the next pair; then normalize with
                # 1/den = exp(-ln(den)) computed on ScalarE (vector.reciprocal
                # on a single-partition row is ~8x-slow serial DVE work).
                den2 = attn.tile([1, 1024], BF16, tag="den2", bufs=1,
                                 name="den2")
                cu = attn.tile([64, 512], BF16, tag="cu", bufs=2, name="cu")
                nc.vector.tensor_copy(ctxT[j][0:HD, :], pcx0[:HD, :])
                nc.vector.tensor_copy(den2[:, 0:512], pcx0[HD : HD + 1, :])
                nc.vector.tensor_copy(cu[:], pcx1[:HD, :])
                nc.vector.tensor_copy(den2[:, 512:1024], pcx1[HD : HD + 1, :])
                lnd = attn.tile([1, 1024], F32, tag="lnd", bufs=1, name="lnd")
                nc.scalar.activation(lnd[:], den2[:], AF.Ln)
                rdenb2 = attn.tile([1, 1024], BF16, tag="rdenb2", bufs=1,
                                   name="rdenb2")
                nc.scalar.activation(rdenb2[:], lnd[:], AF.Exp, scale=-1.0)
                bcs0 = attn.tile([64, 512], BF16, tag="bcs", bufs=2,
                                 name="bcs")
                nc.gpsimd.partition_broadcast(bcs0[:], rdenb2[:, 0:512])
                nc.vector.tensor_mul(ctxT[j][0:HD, :], ctxT[j][0:HD, :],
                                     bcs0[:])
                bcs1 = attn.tile([64, 512], BF16, tag="bcs", bufs=2,
                                 name="bcs")
                nc.gpsimd.partition_broadcast(bcs1[:], rdenb2[:, 512:1024])
                nc.vector.tensor_mul(ctxT[j][HD:128, :], cu[:], bcs1[:])

            # output projection + residual -> xres (f32)
            for t in (range(QT) if stop_stage not in ("qkv", "heads") else range(0)):
                for hf in range(2):
                    ps = ps_mm.tile([128, 512], F32, tag="mm", name="ps_o")
                    for d in range(DC):
                        nc.tensor.matmul(
                            ps[:, :384],
                            ctxT[d][:, t * 128 : (t + 1) * 128],
                            wo_sb[d][:, hf * 384 : (hf + 1) * 384],
                            start=(d == 0), stop=(d == DC - 1))
                    nc.vector.tensor_add(xres[t][:, hf * 384 : (hf + 1) * 384],
                                         ps[:, :384],
                                         xq_sb[t][:, hf * 384 : (hf + 1) * 384])

            if stop_stage == "heads":
                for t in range(QT):
                    nc.sync.dma_start(out_d[t * 128 : (t + 1) * 128, :],
                                      xq_sb[t][:])
            if stop_stage == "attn":
                for t in range(QT):
                    nc.sync.dma_start(out_d[t * 128 : (t + 1) * 128, :],
                                      xres[t][:])
            # LN2 + transpose into h2T (kept in glob for MoE phase)
            do_rest = stop_stage in ("full", "gates")
            if do_rest:
                xn2, ln2mu, ln2rstd = _layernorm_tiles(
                    nc, small, junkp, xres, QT, xnp, "ln2", eps_sb)
                h2f8 = glob.tile([128, DC, 512], FP8, tag="h2f8", name="h2f8")
                for d in range(DC):
                    dt_ = glob.tile([128, 512], BF16, tag="h2_T", bufs=DC,
                                    name="h2_T")
                    ps = ps_tr.tile([128, 1024], BF16, tag="big", bufs=2,
                                    name="tr_ps")
                    for t in range(QT):
                        nc.tensor.transpose(ps[:, t * 128 : (t + 1) * 128],
                                            xn2[t][:, d * 128 : (d + 1) * 128],
                                            ident[:])
                    nc.vector.tensor_copy(dt_[:], ps[:, :512])
                    nc.scalar.copy(h2f8[:, d, :], ps[:, :512])
                    h2T.append(dt_)

            # gate logits + top-2 softmax gates
            pp_cm.__exit__(None, None, None)

        if stop_stage == "gates":
            for t in range(QT):
                nc.sync.dma_start(out_d[t * 128 : (t + 1) * 128, :], xres[t][:])
        # ---- MoE phase (fp8 DoubleRow matmuls) -----------------------------
        # W1/W2 are host-quantized to fp8e4 scaled by WSCALE; the 1/WSCALE is
        # folded into the gelu input scale (W1) and a pre-scaled gates vector
        # (W2). Contraction runs in K=256 DoubleRow chunks: both operands are
        # [128, 2, N] with the pair index selecting the two K-subchunks.
        with (
            tc.tile_pool(name="hidp", bufs=14) as hidp,
            tc.tile_pool(name="w2p", bufs=13) as w2p,
        ):
            gtp_cm = tc.tile_pool(name="gtp", bufs=1)
            gtp = gtp_cm.__enter__()
            gates32 = [glob.tile([128, E], F32, tag="gates32", bufs=QT,
                                 name="gates32") for _ in range(QT)]
            # Gate logits in full f32 on DVE (bf16 matmul noise flips the
            # top-2 selection on knife-edge tokens):
            #   logit[t, e] = rstd_t * (xres_t . wg_e - mu_t * sum(wg_e)) + bias_e
            if do_rest:
                wgb = [gtp.tile([128, D], F32, tag="wgb", bufs=E, name="wgb")
                       for _ in range(E)]
                for e_ in range(E):
                    nc.sync.dma_start(wgb[e_][:],
                                      wgb_d[:, e_ * D : (e_ + 1) * D])
            for t in (range(QT) if do_rest else range(0)):
                glog = gtp.tile([128, E], F32, tag="glog", bufs=2, name="glog")
                for e_ in range(E):
                    jr = gtp.tile([128, D], F32, tag="jr32", bufs=2,
                                 name="jr32")
                    nc.vector.tensor_mul(jr[:], xres[t][:], wgb[e_][:])
                    dot = gtp.tile([128, 1], F32, tag="gdot", bufs=2,
                                    name="gdot")
                    nc.vector.reduce_sum(dot[:], jr[:], axis=AX.X)
                    mterm = gtp.tile([128, 1], F32, tag="gmt", bufs=2,
                                      name="gmt")
                    nc.vector.tensor_scalar(
                        out=mterm[:], in0=ln2mu[:, t : t + 1],
                        scalar1=float(meta["swg"][e_]), scalar2=None,
                        op0=ALU.mult)
                    nc.vector.tensor_sub(dot[:], dot[:], mterm[:])
                    nc.vector.tensor_mul(dot[:], dot[:],
                                         ln2rstd[:, t : t + 1])
                    nc.vector.tensor_add(glog[:, e_ : e_ + 1], dot[:],
                                         bias4b[:, e_ : e_ + 1])
                m1 = gtp.tile([128, 1], F32, tag="m1", bufs=2, name="m1")
                nc.vector.reduce_max(m1[:], glog[:], axis=AX.X)
                eq1 = gtp.tile([128, E], F32, tag="eq1", bufs=2, name="eq1")
                nc.vector.tensor_scalar(out=eq1[:], in0=glog[:], scalar1=m1[:],
                                        scalar2=None, op0=ALU.is_equal)
                big = gtp.tile([128, E], F32, tag="big", bufs=2, name="big")
                nc.vector.tensor_scalar(out=big[:], in0=eq1[:], scalar1=1e30,
                                        scalar2=None, op0=ALU.mult)
                msk = gtp.tile([128, E], F32, tag="msk", bufs=2, name="msk")
                nc.vector.tensor_sub(msk[:], glog[:], big[:])
                m2 = gtp.tile([128, 1], F32, tag="m2", bufs=2, name="m2")
                nc.vector.reduce_max(m2[:], msk[:], axis=AX.X)
                eq2 = gtp.tile([128, E], F32, tag="eq2", bufs=2, name="eq2")
                nc.vector.tensor_scalar(out=eq2[:], in0=msk[:], scalar1=m2[:],
                                        scalar2=None, op0=ALU.is_equal)
                nm1 = gtp.tile([128, 1], F32, tag="nm1", bufs=2, name="nm1")
                nc.vector.tensor_scalar(out=nm1[:], in0=m1[:], scalar1=-1.0,
                                        scalar2=None, op0=ALU.mult)
                dx = gtp.tile([128, 1], F32, tag="dx", bufs=2, name="dx")
                nc.scalar.activation(dx[:], m2[:], AF.Exp, bias=nm1[:], scale=1.0)
                sden = gtp.tile([128, 1], F32, tag="sden", bufs=2, name="sden")
                nc.vector.tensor_scalar(out=sden[:], in0=dx[:], scalar1=1.0,
                                        scalar2=None, op0=ALU.add)
                w1s = gtp.tile([128, 1], F32, tag="w1s", bufs=2, name="w1s")
                nc.vector.reciprocal(w1s[:], sden[:])
                w2s = gtp.tile([128, 1], F32, tag="w2s", bufs=2, name="w2s")
                nc.vector.tensor_mul(w2s[:], dx[:], w1s[:])
                ga = gtp.tile([128, E], F32, tag="ga", bufs=2, name="ga")
                nc.vector.tensor_scalar(out=ga[:], in0=eq1[:], scalar1=w1s[:],
                                        scalar2=None, op0=ALU.mult)
                gb = gtp.tile([128, E], F32, tag="gb", bufs=2, name="gb")
                nc.vector.tensor_scalar(out=gb[:], in0=eq2[:], scalar1=w2s[:],
                                        scalar2=None, op0=ALU.mult)
                nc.vector.tensor_add(gates[t][:], ga[:], gb[:])
                nc.vector.tensor_scalar(out=gates32[t][:], in0=gates[t][:],
                                        scalar1=1.0 / WSCALE, scalar2=None,
                                        op0=ALU.mult)
            for e in (range(E) if stop_stage == "full" else range(0)):
                w2_sb = []
                for fp in range(FC // 2):
                    wt_ = w2p.tile([128, 2, D], FP8, tag="w2f", bufs=13,
                                   name="w2f")
                    nc.sync.dma_start(wt_[:], w2_d[e, fp].rearrange(
                        "p (i d) -> p i d", i=2))
                    w2_sb.append(wt_)

                hidT = []
                for fcg in range(DC):
                    w1g = w1p.tile([128, 3, 2, 512], FP8, tag="w1g", bufs=2,
                                   name="w1g")
                    nc.sync.dma_start(w1g[:], w1_d[e, fcg].rearrange(
                        "p (c i f) -> p c i f", c=3, i=2))
                    for fcm in range(4):
                        fc = fcg * 4 + fcm
                        ps = ps_mm.tile([128, 512], F32, tag="mm", name="ps_h")
                        for c in range(3):
                            nc.tensor.matmul(
                                ps[:],
                                w1g[:, c, :, fcm * 128 : (fcm + 1) * 128],
                                h2f8[:, 2 * c : 2 * c + 2, :],
                                start=(c == 0), stop=(c == 2),
                                perf_mode=mybir.MatmulPerfMode.DoubleRow)
                        if fc % 2 == 0:
                            hpair = hidp.tile([128, 2, 512], FP8, tag="hid",
                                              bufs=14, name="hid")
                            hidT.append(hpair)
                        nc.scalar.activation(
                            hidT[fc // 2][:, fc % 2, :], ps[:],
                            AF.Gelu_apprx_tanh,
                            bias=b1t_sb[:, e * FC + fc : e * FC + fc + 1],
                            scale=1.0 / WSCALE)

                for t in range(QT):
                    pya = ps_mm.tile([128, 512], F32, tag="mm", name="ps_ya")
                    pyb = ps_cx.tile([128, 512], F32, tag="cx", name="ps_yb")
                    for fp in range(FC // 2):
                        nc.tensor.matmul(
                            pya[:, :384],
                            hidT[fp][:, :, t * 128 : (t + 1) * 128],
                            w2_sb[fp][:, :, 0:384],
                            start=(fp == 0), stop=(fp == FC // 2 - 1),
                            perf_mode=mybir.MatmulPerfMode.DoubleRow)
                        nc.tensor.matmul(
                            pyb[:, :384],
                            hidT[fp][:, :, t * 128 : (t + 1) * 128],
                            w2_sb[fp][:, :, 384:768],
                            start=(fp == 0), stop=(fp == FC // 2 - 1),
                            perf_mode=mybir.MatmulPerfMode.DoubleRow)
                    for hf, py in ((0, pya), (1, pyb)):
                        ys = junkp.tile([128, 384], F32, tag="ys", bufs=2,
                                        name="ys")
                        nc.scalar.activation(ys[:], py[:, :384], AF.Identity,
                                             bias=0.0,
                                             scale=gates32[t][:, e : e + 1])
                        nc.vector.tensor_add(
                            xres[t][:, hf * 384 : (hf + 1) * 384],
                            xres[t][:, hf * 384 : (hf + 1) * 384], ys[:])
                    if e == E - 1:
                        nc.sync.dma_start(out_d[t * 128 : (t + 1) * 128, :],
                                          xres[t][:])
            gtp_cm.__exit__(None, None, None)

    nc.compile()
    return nc


def prepare_inputs(inputs):
    x = np.asarray(inputs["x"], np.float32)
    ln1_g = np.asarray(inputs["ln1_g"], np.float32)
    ln1_b = np.asarray(inputs["ln1_b"], np.float32)
    Wq = np.asarray(inputs["Wq"], np.float32)
    Wk = np.asarray(inputs["Wk"], np.float32)
    Wv = np.asarray(inputs["Wv"], np.float32)
    Wo = np.asarray(inputs["Wo"], np.float32)
    Wt = np.asarray(inputs["Wt"], np.float32)
    bt = np.asarray(inputs["bt"], np.float32)
    ln2_g = np.asarray(inputs["ln2_g"], np.float32)
    ln2_b = np.asarray(inputs["ln2_b"], np.float32)
    Wg = np.asarray(inputs["Wg"], np.float32)
    W1 = np.asarray(inputs["W1"], np.float32)
    b1 = np.asarray(inputs["b1"], np.float32)
    W2 = np.asarray(inputs["W2"], np.float32)
    b2 = np.asarray(inputs["b2"], np.float32)

    bv = ln1_b @ Wv
    if np.any(b2) or np.any(bv):
        raise NotImplementedError("nonzero b2 / ln1_b@Wv path not implemented")

    wq = (ln1_g[:, None] * Wq).astype(BF16NP)
    wk = (ln1_g[:, None] * Wk).astype(BF16NP)
    wv = (ln1_g[:, None] * Wv).astype(BF16NP)
    wo = Wo.astype(BF16NP)
    bq = ln1_b @ Wq
    bk = ln1_b @ Wk
    bqk = np.concatenate([bq.reshape(DC, 128).T, bk.reshape(DC, 128).T],
                         axis=1).astype(np.float32)
    has_qkbias = bool(np.any(bqk))

    wg = (ln2_g[:, None] * Wg).astype(BF16NP)
    wg_r = np.ascontiguousarray(
        wg.reshape(DC, 128, E).transpose(1, 0, 2).reshape(128, DC * E))
    bg = (ln2_b @ Wg).reshape(1, E).astype(np.float32)
    wg32 = (ln2_g[:, None] * Wg).astype(np.float32)
    wg_bcast = np.ascontiguousarray(np.broadcast_to(
        wg32.T.reshape(1, E * D), (128, E * D)))
    swg = wg32.sum(axis=0)

    # fp8 DoubleRow layout for W1: [e, fcg, p, (c i fgrp)] where the
    # contraction index is d = c*256 + i*128 + p.
    w1 = ((ln2_g[None, :, None] * W1) * WSCALE).astype(FP8NP)
    w1r = np.ascontiguousarray(
        w1.reshape(E, 3, 2, 128, DC, 512).transpose(0, 4, 3, 1, 2, 5)
        .reshape(E, DC, 128, 3 * 2 * 512))
    b1_tot = (b1 + np.einsum("d,edf->ef", ln2_b, W1)).astype(np.float32)
    b1t = np.ascontiguousarray(
        b1_tot.reshape(E, FC, 128).transpose(2, 0, 1).reshape(128, E * FC))
    # fp8 DoubleRow layout for W2: [e, fp, p, (i d)], contraction index
    # f = fp*256 + i*128 + p.
    w2 = (W2 * WSCALE).astype(FP8NP)
    w2r = np.ascontiguousarray(
        w2.reshape(E, FC // 2, 2, 128, D).transpose(0, 1, 3, 2, 4)
        .reshape(E, FC // 2, 128, 2 * D))
    wt_t = np.ascontiguousarray(Wt.T).astype(BF16NP).reshape(1, 2 * D)

    meta = {"bt0": float(bt[0]), "bt1": float(bt[1]),
            "has_qkbias": has_qkbias, "swg": [float(v) for v in swg]}
    shared = {
        "wq": wq, "wk": wk, "wv": wv, "wo": wo, "wt_t": wt_t,
        "wg_r": wg_r, "bg": bg, "w1r": w1r, "w2r": w2r, "b1t": b1t,
        "bqk": bqk, "wg_bcast": wg_bcast,
    }
    in_maps = []
    for i in range(N_CORES):
        b, half = i // 2, i % 2
        m = dict(shared)
        m["x_kv"] = np.ascontiguousarray(x[b])
        m["x_q"] = np.ascontiguousarray(x[b, half * NQ : (half + 1) * NQ])
        in_maps.append(m)
    return meta, in_maps


def kernel(**inputs):
    meta, in_maps = prepare_inputs(inputs)
    key = ("v1", meta["has_qkbias"], meta["bt0"], meta["bt1"], tuple(meta["swg"]))
    if key not in _CACHED:
        _CACHED[key] = build_program(meta)
    nc = _CACHED[key]

    res = run_bass_kernel_spmd(nc, in_maps, list(range(N_CORES)),
                               trace=bool(inputs.get("_trace", False)))
    out = np.empty((B, S, D), np.float32)
    for i in range(N_CORES):
        b, half = i // 2, i % 2
        out[b, half * NQ : (half + 1) * NQ] = res.results[i]["out"]
    if inputs.get("_want_time", False):
        return out, res
    return out


# revision 45
# speedup vs baseline: 1.0467x; 1.0077x over previous
"""Trainium2 Bass kernel for a transformer block with attention + top-2-of-4 MoE.

Problem (B=4, S=1024, D=768, H=12, E=4, DF=3072, TOPK=2):
  pooled task-routing bias -> pre-norm MHA with residual -> pre-norm top-2 MoE
  with routing bias, residual.

Sharding: 8 cores, token-parallel. Core i handles batch b=i//2, sequence half
i%2 (512 query tokens). Each core gets the full 1024-token sequence of its
batch (x_kv) to compute K/V and the pooled routing bias locally; no
collectives are needed. All weights are replicated, pre-cast to bf16 on the
host, with LayerNorm gammas/betas folded into the consuming weight matrices.

Attention runs in bf16 on the TensorEngine with fp32 PSUM accumulation;
LayerNorm statistics, softmax denominators and residuals stay fp32. Softmax
runs without max-subtraction (logits are O(1) for this problem's weight
scale), which lets scores be computed directly in [key, query] layout so no
attention-probability transposes are needed: the denominator comes from an
appended ones-column in the value tensor and 1/den = exp(-ln(den)) is applied
when evicting per-head context (vector.reciprocal on a [1, N] row is ~8x-slow
single-lane DVE work). The two heads of each pair occupy PE row groups 0 and
64 so their K=64 score matmuls run concurrently (a lone K=64 matmul streams
at half rate).

The MoE expert FFNs run as fp8e4m3 DoubleRow matmuls (weights host-quantized
with a x32 scale folded back out via the gelu input scale and a pre-scaled
gates vector), ~1.4x the bf16 TensorEngine throughput. Gate logits are
computed in full fp32 on the VectorEngine (overlapping expert matmuls) since
bf16 logit noise flips the top-2 selection on knife-edge tokens.
"""

import contextlib

import numpy as np
import ml_dtypes

import concourse.bass as bass
import concourse.bacc as bacc
import concourse.mybir as mybir
import concourse.tile as tile
from concourse import bass_isa, masks
from concourse.bass_utils import run_bass_kernel_spmd

B, S, D, H, E, DF = 4, 1024, 768, 12, 4, 3072
HD = D // H          # 64
NQ = S // 2          # 512 tokens owned per core
NKV = S
N_CORES = 8
DC = D // 128        # 6 d-chunks
FC = DF // 128       # 24 f-chunks
QT = NQ // 128       # 4 query token tiles
KT = NKV // 128      # 8 kv token tiles
EPS = 1e-5

F32 = mybir.dt.float32
BF16 = mybir.dt.bfloat16
AF = mybir.ActivationFunctionType
ALU = mybir.AluOpType
AX = mybir.AxisListType
BF16NP = ml_dtypes.bfloat16
FP8 = mybir.dt.float8e4
FP8NP = ml_dtypes.float8_e4m3
WSCALE = 32.0

_CACHED = {}


def _layernorm_tiles(nc, small, junkp, x_tiles, n_tiles, xn_pool, name,
                     eps_ap=None):
    """LN over [128, D] f32 tiles -> bf16 normalized tiles (gamma/beta folded
    into downstream weights on the host). Returns list of bf16 [128, D] tiles."""
    stats_s = small.tile([128, n_tiles], F32, tag=f"{name}_s", name=f"{name}_s")
    stats_q = small.tile([128, n_tiles], F32, tag=f"{name}_q", name=f"{name}_q")
    for t in range(n_tiles):
        nc.vector.reduce_sum(stats_s[:, t : t + 1], x_tiles[t][:], axis=AX.X)
        j = junkp.tile([128, D], BF16, tag="junk", bufs=2, name="junk")
        nc.scalar.activation(j[:], x_tiles[t][:], AF.Square,
                             accum_out=stats_q[:, t : t + 1])
    mu = small.tile([128, n_tiles], F32, tag=f"{name}_mu", name=f"{name}_mu")
    var = small.tile([128, n_tiles], F32, tag=f"{name}_var", name=f"{name}_var")
    nc.vector.tensor_scalar(out=mu[:], in0=stats_s[:], scalar1=1.0 / D,
                            scalar2=None, op0=ALU.mult)
    nc.vector.tensor_scalar(out=var[:], in0=stats_q[:], scalar1=1.0 / D,
                            scalar2=None, op0=ALU.mult)
    mu2 = small.tile([128, n_tiles], F32, tag=f"{name}_mu2", name=f"{name}_mu2")
    nc.vector.tensor_mul(mu2[:], mu[:], mu[:])
    nc.vector.tensor_sub(var[:], var[:], mu2[:])
    lnv = small.tile([128, n_tiles], F32, tag=f"{name}_lnv", name=f"{name}_lnv")
    nc.scalar.activation(lnv[:], var[:], AF.Ln, bias=eps_ap[:], scale=1.0)
    rstd = small.tile([128, n_tiles], F32, tag=f"{name}_rstd", name=f"{name}_rstd")
    nc.scalar.activation(rstd[:], lnv[:], AF.Exp, scale=-0.5)
    nmr = small.tile([128, n_tiles], F32, tag=f"{name}_nmr", name=f"{name}_nmr")
    nc.vector.tensor_mul(nmr[:], mu[:], rstd[:])
    nc.vector.tensor_scalar(out=nmr[:], in0=nmr[:], scalar1=-1.0, scalar2=None,
                            op0=ALU.mult)
    xn_tiles = []
    for t in range(n_tiles):
        xn = xn_pool.tile([128, D], BF16, tag=f"{name}_xn", bufs=n_tiles,
                          name=f"{name}_xn")
        nc.scalar.activation(xn[:], x_tiles[t][:], AF.Identity,
                             bias=nmr[:, t : t + 1], scale=rstd[:, t : t + 1])
        xn_tiles.append(xn)
    return xn_tiles, mu, rstd


def _transpose_to(nc, ps_tr, ident, xn_tiles, dst_pool, n_tok_tiles, name,
                  dst_bufs=6):
    """Transpose token-major bf16 tiles [128tok, D] into d-major tiles
    [128d, n_tok_tiles*128]. Returns list of DC tiles."""
    dst = []
    for d in range(DC):
        dt_ = dst_pool.tile([128, n_tok_tiles * 128], BF16, tag=f"{name}_T",
                            bufs=dst_bufs, name=f"{name}_T")
        ps = ps_tr.tile([128, 1024], BF16, tag="big", bufs=2, name="tr_ps")
        for t in range(n_tok_tiles):
            nc.tensor.transpose(ps[:, t * 128 : (t + 1) * 128],
                                xn_tiles[t][:, d * 128 : (d + 1) * 128], ident[:])
        nc.vector.tensor_copy(dt_[:], ps[:, : n_tok_tiles * 128])
        dst.append(dt_)
    return dst


def build_program(meta, stop_stage='full'):
    nc = bacc.Bacc("TRN2", target_bir_lowering=False, debug=False,
                   num_devices=N_CORES)

    # ---- DRAM parameters --------------------------------------------------
    xq_d = nc.declare_dram_parameter("x_q", [NQ, D], F32, isOutput=False)
    xkv_d = nc.declare_dram_parameter("x_kv", [NKV, D], F32, isOutput=False)
    wq_d = nc.declare_dram_parameter("wq", [D, D], BF16, isOutput=False)
    wk_d = nc.declare_dram_parameter("wk", [D, D], BF16, isOutput=False)
    wv_d = nc.declare_dram_parameter("wv", [D, D], BF16, isOutput=False)
    wo_d = nc.declare_dram_parameter("wo", [D, D], BF16, isOutput=False)
    wtt_d = nc.declare_dram_parameter("wt_t", [1, 2 * D], BF16, isOutput=False)
    wg_d = nc.declare_dram_parameter("wg_r", [128, DC * E], BF16, isOutput=False)
    bg_d = nc.declare_dram_parameter("bg", [1, E], F32, isOutput=False)
    wgb_d = nc.declare_dram_parameter("wg_bcast", [128, E * D], F32, isOutput=False)
    w1_d = nc.declare_dram_parameter("w1r", [E, DC, 128, 3 * 2 * 512], FP8, isOutput=False)
    w2_d = nc.declare_dram_parameter("w2r", [E, FC // 2, 128, 2 * D], FP8, isOutput=False)
    b1_d = nc.declare_dram_parameter("b1t", [128, E * FC], F32, isOutput=False)
    bqk_d = nc.declare_dram_parameter("bqk", [128, 2 * DC], F32, isOutput=False)
    out_d = nc.declare_dram_parameter("out", [NQ, D], F32, isOutput=True)

    has_qkbias = meta["has_qkbias"]
    bt0, bt1 = meta["bt0"], meta["bt1"]

    with tile.TileContext(nc) as tc, contextlib.ExitStack() as ctx:
        glob = ctx.enter_context(tc.tile_pool(name="glob", bufs=1))
        small = ctx.enter_context(tc.tile_pool(name="small", bufs=1))
        junkp = ctx.enter_context(tc.tile_pool(name="junkp", bufs=2))

        ps_tr = ctx.enter_context(tc.tile_pool(name="ps_tr", bufs=2, space="PSUM"))
        ps_mm = ctx.enter_context(tc.tile_pool(name="ps_mm", bufs=2, space="PSUM"))
        ps_cx = ctx.enter_context(tc.tile_pool(name="ps_cx", bufs=2, space="PSUM"))
        ps_s = ps_tr  # scores share the 2x [128,1024]-sized slots with transposes

        ident = glob.tile([128, 128], BF16, tag="ident", name="ident")
        masks.make_identity(nc, ident[:])
        eps_sb = glob.tile([128, 1], F32, tag="eps", name="eps_sb")
        nc.vector.memset(eps_sb[:], EPS)

        xq_sb = []
        for t in range(QT):
            xt = glob.tile([128, D], F32, tag="xq", bufs=QT, name="xq")
            nc.sync.dma_start(xt[:], xq_d[t * 128 : (t + 1) * 128, :])
            xq_sb.append(xt)

        # MoE weight streaming pools (w1 global so expert 0 prefetches early)
        w1p = ctx.enter_context(tc.tile_pool(name="w1p", bufs=2))
        b1t_sb = glob.tile([128, E * FC], F32, tag="b1t", name="b1t")
        nc.sync.dma_start(b1t_sb[:], b1_d[:])
        wg_sb = glob.tile([128, DC, E], BF16, tag="wg", name="wg")
        nc.sync.dma_start(wg_sb[:], wg_d[:].rearrange("p (d e) -> p d e", e=E))
        bg_sb = glob.tile([1, E], F32, tag="bg", name="bg")
        nc.sync.dma_start(bg_sb[:], bg_d[:])

        xres = [glob.tile([128, D], F32, tag="xres", bufs=QT, name="xres")
                for _ in range(QT)]
        gates = [glob.tile([128, E], F32, tag="gates", bufs=QT, name="gates")
                 for _ in range(QT)]
        bias4 = glob.tile([1, E], F32, tag="bias4", name="bias4")
        h2T = []

        # ---- attention phase ---------------------------------------------
        with (
            tc.tile_pool(name="attn", bufs=1) as attn,
            tc.tile_pool(name="xnp", bufs=8) as xnp,
        ):
            wsub_cm = tc.tile_pool(name="wsub", bufs=1)
            wsub = wsub_cm.__enter__()
            xkv_sb = []
            for t in range(KT):
                xt = attn.tile([128, D], F32, tag="xkv", bufs=KT, name="xkv")
                nc.sync.dma_start(xt[:], xkv_d[t * 128 : (t + 1) * 128, :])
                xkv_sb.append(xt)
            wq_sb, wk_sb, wv_sb, wo_sb = [], [], [], []
            for (wd, lst, tg) in ((wq_d, wq_sb, "wq"), (wk_d, wk_sb, "wk"),
                                  (wv_d, wv_sb, "wv")):
                for d in range(DC):
                    wt_ = wsub.tile([128, D], BF16, tag=tg, bufs=DC, name=tg)
                    nc.sync.dma_start(wt_[:], wd[d * 128 : (d + 1) * 128, :])
                    lst.append(wt_)
            for d in range(DC):
                wt2_ = attn.tile([128, D], BF16, tag="wo", bufs=DC, name="wo")
                nc.sync.dma_start(wt2_[:], wo_d[d * 128 : (d + 1) * 128, :])
                wo_sb.append(wt2_)
            wtt_sb = attn.tile([1, 2, D], BF16, tag="wtt", name="wtt")
            nc.sync.dma_start(wtt_sb[:], wtt_d[:].rearrange("p (j d) -> p j d", j=2))
            bqk_sb = attn.tile([128, 2 * DC], F32, tag="bqk", name="bqk")
            nc.sync.dma_start(bqk_sb[:], bqk_d[:])

            # pooled routing bias: pooled = mean_t x_kv; tl = pooled @ Wt + bt
            # (partition sum on GpSimd to keep it off the TensorEngine)
            pooled = attn.tile([1, D], F32, tag="pooled", name="pooled")
            psum_x = junkp.tile([128, D], BF16, tag="junk", bufs=2,
                                name="junk")
            nc.vector.tensor_add(psum_x[:], xkv_sb[0][:], xkv_sb[1][:])
            for t in range(2, KT):
                nc.vector.tensor_add(psum_x[:], psum_x[:], xkv_sb[t][:])
            pall = junkp.tile([128, D], BF16, tag="junk", bufs=2, name="junk")
            nc.gpsimd.partition_all_reduce(pall[:], psum_x[:], channels=128,
                                           reduce_op=bass_isa.ReduceOp.add)
            nc.vector.tensor_scalar(out=pooled[:], in0=pall[0:1, :],
                                    scalar1=1.0 / NKV, scalar2=None,
                                    op0=ALU.mult)
            tl = attn.tile([1, 2], F32, tag="tl", name="tl")
            for j, btj in ((0, bt0), (1, bt1)):
                jrow = junkp.tile([128, D], BF16, tag="junk", bufs=2, name="junk")
                nc.vector.tensor_mul(jrow[0:1, :], pooled[:], wtt_sb[:, j, :])
                nc.vector.reduce_sum(tl[:, j : j + 1], jrow[0:1, :], axis=AX.X)
                nc.vector.tensor_scalar(out=tl[:, j : j + 1],
                                        in0=tl[:, j : j + 1],
                                        scalar1=float(btj), scalar2=None,
                                        op0=ALU.add)
            for e_, j_ in ((0, 0), (1, 1), (2, 0), (3, 1)):  # DOMAIN_MAP
                nc.vector.tensor_copy(bias4[:, e_ : e_ + 1], tl[:, j_ : j_ + 1])
            nc.vector.tensor_add(bias4[:], bias4[:], bg_sb[:])
            bias4b = glob.tile([128, E], F32, tag="bias4b", name="bias4b")
            nc.gpsimd.partition_broadcast(bias4b[:], bias4[:])

            # LN1 -> transposed bf16 activations
            xn_kv, _, _ = _layernorm_tiles(nc, small, junkp, xkv_sb, KT, xnp, "ln1kv", eps_sb)
            xn_q, _, _ = _layernorm_tiles(nc, small, junkp, xq_sb, QT, xnp, "ln1q", eps_sb)
            hkvT = _transpose_to(nc, ps_tr, ident, xn_kv, attn, KT, "hkv")
            hqT = _transpose_to(nc, ps_tr, ident, xn_q, attn, QT, "hq")

            # Q/K projections -> d-major qT [D, NQ], kT [D, NKV]
            qT = [attn.tile([128, NQ], BF16, tag="qT", bufs=DC, name="qT")
                  for _ in range(DC)]
            kT = [attn.tile([128, NKV], BF16, tag="kT", bufs=DC, name="kT")
                  for _ in range(DC)]
            for m in range(DC):
                ps = ps_mm.tile([128, 512], F32, tag="mm", name="ps_q")
                for d in range(DC):
                    nc.tensor.matmul(ps[:], wq_sb[d][:, m * 128 : (m + 1) * 128],
                                     hqT[d][:], start=(d == 0), stop=(d == DC - 1))
                if has_qkbias:
                    nc.scalar.activation(qT[m][:], ps[:], AF.Identity,
                                         bias=bqk_sb[:, m : m + 1], scale=1.0)
                else:
                    nc.vector.tensor_copy(qT[m][:], ps[:])
                for hf in range(2):
                    ps2 = ps_mm.tile([128, 512], F32, tag="mm", name="ps_k")
                    for d in range(DC):
                        nc.tensor.matmul(
                            ps2[:], wk_sb[d][:, m * 128 : (m + 1) * 128],
                            hkvT[d][:, hf * 512 : (hf + 1) * 512],
                            start=(d == 0), stop=(d == DC - 1))
                    if has_qkbias:
                        nc.scalar.activation(kT[m][:, hf * 512 : (hf + 1) * 512],
                                             ps2[:], AF.Identity,
                                             bias=bqk_sb[:, DC + m : DC + m + 1],
                                             scale=1.0)
                    else:
                        nc.vector.tensor_copy(kT[m][:, hf * 512 : (hf + 1) * 512],
                                              ps2[:])

            # V projection (token-major) into v_aug [128, H, HD+1], ones col
            v_aug = [attn.tile([128, H, HD + 1], BF16, tag="vaug", bufs=KT,
                               name="vaug") for _ in range(KT)]
            for t in range(KT):
                for hf in range(2):
                    ps = ps_mm.tile([128, 512], F32, tag="mm", name="ps_v")
                    for d in range(DC):
                        nc.tensor.matmul(
                            ps[:, :384],
                            hkvT[d][:, t * 128 : (t + 1) * 128],
                            wv_sb[d][:, hf * 384 : (hf + 1) * 384],
                            start=(d == 0), stop=(d == DC - 1))
                    nc.vector.tensor_copy(
                        v_aug[t][:, hf * 6 : (hf + 1) * 6, :HD],
                        ps[:, :384].rearrange("p (h x) -> p h x", h=6))
                nc.vector.memset(v_aug[t][:, :, HD : HD + 1], 1.0)
            wsub_cm.__exit__(None, None, None)
            pp_cm = tc.tile_pool(name="pp", bufs=6)
            pp = pp_cm.__enter__()

            if stop_stage == "qkv":
                for t in range(QT):
                    nc.sync.dma_start(out_d[t * 128 : (t + 1) * 128, :],
                                      xq_sb[t][:])
            # head-pair attention: the two heads of a pair live in row
            # groups 0 and 64 of the same qT/kT tile, so their K=64 score
            # matmuls execute concurrently on disjoint PE row groups.
            ctxT = [attn.tile([128, NQ], BF16, tag="ctxT", bufs=DC, name="ctxT")
                    for _ in range(DC)]
            pairs = range(H // 2) if stop_stage != "qkv" else range(0)
            def _emit_scores(j):
                p_sb = []
                for c in range(KT):
                    pss = ps_s.tile([128, 1024], F32, tag="big", bufs=2,
                                    name="ps_s")
                    nc.tensor.matmul(
                        pss[:, 0:512],
                        kT[j][0:HD, c * 128 : (c + 1) * 128],
                        qT[j][0:HD, :], start=True, stop=True)
                    nc.tensor.matmul(
                        pss[:, 512:1024],
                        kT[j][HD:128, c * 128 : (c + 1) * 128],
                        qT[j][HD:128, :], start=True, stop=True)
                    pc = pp.tile([128, 1024], BF16, tag="p", bufs=9, name="p")
                    nc.scalar.activation(pc[:], pss[:], AF.Exp,
                                         scale=float(1.0 / np.sqrt(HD)))
                    p_sb.append(pc)
                return p_sb

            def _emit_scores_interleaved(j, prev_j, prev_p, prev_cx):
                # chunk-interleave pair j's score matmuls with pair prev_j's
                # AV matmuls so PE work is available while ScalarE streams
                # the exps for pair j.
                p_sb = []
                pcx0, pcx1 = prev_cx
                for c in range(KT):
                    nc.tensor.matmul(pcx0[:], v_aug[c][:, 2 * prev_j, :],
                                     prev_p[c][:, 0:512],
                                     start=(c == 0), stop=(c == KT - 1))
                    nc.tensor.matmul(pcx1[:], v_aug[c][:, 2 * prev_j + 1, :],
                                     prev_p[c][:, 512:1024],
                                     start=(c == 0), stop=(c == KT - 1))
                    pss = ps_s.tile([128, 1024], F32, tag="big", bufs=2,
                                    name="ps_s")
                    nc.tensor.matmul(
                        pss[:, 0:512],
                        kT[j][0:HD, c * 128 : (c + 1) * 128],
                        qT[j][0:HD, :], start=True, stop=True)
                    nc.tensor.matmul(
                        pss[:, 512:1024],
                        kT[j][HD:128, c * 128 : (c + 1) * 128],
                        qT[j][HD:128, :], start=True, stop=True)
                    pc = pp.tile([128, 1024], BF16, tag="p", bufs=9, name="p")
                    nc.scalar.activation(pc[:], pss[:], AF.Exp,
                                         scale=float(1.0 / np.sqrt(HD)))
                    p_sb.append(pc)
                return p_sb

            def _emit_av(j, p_sb):
                pcx0 = ps_cx.tile([HD + 1, 512], F32, tag="cx", name="ps_cx")
                pcx1 = ps_mm.tile([HD + 1, 512], F32, tag="mm", name="ps_cx1")
                for c in range(KT):
                    nc.tensor.matmul(pcx0[:], v_aug[c][:, 2 * j, :],
                                     p_sb[c][:, 0:512],
                                     start=(c == 0), stop=(c == KT - 1))
                    nc.tensor.matmul(pcx1[:], v_aug[c][:, 2 * j + 1, :],
                                     p_sb[c][:, 512:1024],
                                     start=(c == 0), stop=(c == KT - 1))
                return pcx0, pcx1

            def _emit_norm(j, pcx0, pcx1):
                den2 = attn.tile([1, 1024], BF16, tag="den2", bufs=1,
                                 name="den2")
                cu = attn.tile([64, 512], BF16, tag="cu", bufs=2, name="cu")
                nc.vector.tensor_copy(ctxT[j][0:HD, :], pcx0[:HD, :])
                nc.vector.tensor_copy(den2[:, 0:512], pcx0[HD : HD + 1, :])
                nc.vector.tensor_copy(cu[:], pcx1[:HD, :])
                nc.vector.tensor_copy(den2[:, 512:1024], pcx1[HD : HD + 1, :])
                lnd = attn.tile([1, 1024], F32, tag="lnd", bufs=1, name="lnd")
                nc.scalar.activation(lnd[:], den2[:], AF.Ln)
                rdenb2 = attn.tile([1, 1024], BF16, tag="rdenb2", bufs=1,
                                   name="rdenb2")
                nc.scalar.activation(rdenb2[:], lnd[:], AF.Exp, scale=-1.0)
                bcs0 = attn.tile([64, 512], BF16, tag="bcs", bufs=2,
                                 name="bcs")
                nc.gpsimd.partition_broadcast(bcs0[:], rdenb2[:, 0:512])
                nc.vector.tensor_mul(ctxT[j][0:HD, :], ctxT[j][0:HD, :],
                                     bcs0[:])
                bcs1 = attn.tile([64, 512], BF16, tag="bcs", bufs=2,
                                 name="bcs")
                nc.gpsimd.partition_broadcast(bcs1[:], rdenb2[:, 512:1024])
                nc.vector.tensor_mul(ctxT[j][HD:128, :], cu[:], bcs1[:])

            prev = None
            for j in pairs:
                if prev is None:
                    p_sb = _emit_scores(j)
                else:
                    pj, pp_sb = prev
                    pcx = _emit_av_alloc = None
                    pcx0 = ps_cx.tile([HD + 1, 512], F32, tag="cx",
                                      name="ps_cx")
                    pcx1 = ps_mm.tile([HD + 1, 512], F32, tag="mm",
                                      name="ps_cx1")
                    p_sb = _emit_scores_interleaved(j, pj, pp_sb,
                                                    (pcx0, pcx1))
                    _emit_norm(pj, pcx0, pcx1)
                prev = (j, p_sb)
            if prev is not None:
                pj, pp_sb = prev
                pcx0, pcx1 = _emit_av(pj, pp_sb)
                _emit_norm(pj, pcx0, pcx1)

            # output projection + residual -> xres (f32)
            for t in (range(QT) if stop_stage not in ("qkv", "heads") else range(0)):
                for hf in range(2):
                    ps = ps_mm.tile([128, 512], F32, tag="mm", name="ps_o")
                    for d in range(DC):
                        nc.tensor.matmul(
                            ps[:, :384],
                            ctxT[d][:, t * 128 : (t + 1) * 128],
                            wo_sb[d][:, hf * 384 : (hf + 1) * 384],
                            start=(d == 0), stop=(d == DC - 1))
                    nc.vector.tensor_add(xres[t][:, hf * 384 : (hf + 1) * 384],
                                         ps[:, :384],
                                         xq_sb[t][:, hf * 384 : (hf + 1) * 384])

            if stop_stage == "heads":
                for t in range(QT):
                    nc.sync.dma_start(out_d[t * 128 : (t + 1) * 128, :],
                                      xq_sb[t][:])
            if stop_stage == "attn":
                for t in range(QT):
                    nc.sync.dma_start(out_d[t * 128 : (t + 1) * 128, :],
                                      xres[t][:])
            # LN2 + transpose into h2T (kept in glob for MoE phase)
            do_rest = stop_stage in ("full", "gates")
            if do_rest:
                xn2, ln2mu, ln2rstd = _layernorm_tiles(
                    nc, small, junkp, xres, QT, xnp, "ln2", eps_sb)
                h2f8 = glob.tile([128, DC, 512], FP8, tag="h2f8", name="h2f8")
                for d in range(DC):
                    dt_ = glob.tile([128, 512], BF16, tag="h2_T", bufs=DC,
                                    name="h2_T")
                    ps = ps_tr.tile([128, 1024], BF16, tag="big", bufs=2,
                                    name="tr_ps")
                    for t in range(QT):
                        nc.tensor.transpose(ps[:, t * 128 : (t + 1) * 128],
                                            xn2[t][:, d * 128 : (d + 1) * 128],
                                            ident[:])
                    nc.vector.tensor_copy(dt_[:], ps[:, :512])
                    nc.scalar.copy(h2f8[:, d, :], ps[:, :512])
                    h2T.append(dt_)

            # gate logits + top-2 softmax gates
            pp_cm.__exit__(None, None, None)

        if stop_stage == "gates":
            for t in range(QT):
                nc.sync.dma_start(out_d[t * 128 : (t + 1) * 128, :], xres[t][:])
        # ---- MoE phase (fp8 DoubleRow matmuls) -----------------------------
        # W1/W2 are host-quantized to fp8e4 scaled by WSCALE; the 1/WSCALE is
        # folded into the gelu input scale (W1) and a pre-scaled gates vector
        # (W2). Contraction runs in K=256 DoubleRow chunks: both operands are
        # [128, 2, N] with the pair index selecting the two K-subchunks.
        with (
            tc.tile_pool(name="hidp", bufs=14) as hidp,
            tc.tile_pool(name="w2p", bufs=13) as w2p,
        ):
            gtp_cm = tc.tile_pool(name="gtp", bufs=1)
            gtp = gtp_cm.__enter__()
            gates32 = [glob.tile([128, E], F32, tag="gates32", bufs=QT,
                                 name="gates32") for _ in range(QT)]
            # Gate logits in full f32 on DVE (bf16 matmul noise flips the
            # top-2 selection on knife-edge tokens):
            #   logit[t, e] = rstd_t * (xres_t . wg_e - mu_t * sum(wg_e)) + bias_e
            if do_rest:
                wgb = [gtp.tile([128, D], F32, tag="wgb", bufs=E, name="wgb")
                       for _ in range(E)]
                for e_ in range(E):
                    nc.sync.dma_start(wgb[e_][:],
                                      wgb_d[:, e_ * D : (e_ + 1) * D])
            for t in (range(QT) if do_rest else range(0)):
                glog = gtp.tile([128, E], F32, tag="glog", bufs=2, name="glog")
                for e_ in range(E):
                    jr = gtp.tile([128, D], F32, tag="jr32", bufs=2,
                                 name="jr32")
                    nc.vector.tensor_mul(jr[:], xres[t][:], wgb[e_][:])
                    dot = gtp.tile([128, 1], F32, tag="gdot", bufs=2,
                                    name="gdot")
                    nc.vector.reduce_sum(dot[:], jr[:], axis=AX.X)
                    mterm = gtp.tile([128, 1], F32, tag="gmt", bufs=2,
                                      name="gmt")
                    nc.vector.tensor_scalar(
                        out=mterm[:], in0=ln2mu[:, t : t + 1],
                        scalar1=float(meta["swg"][e_]), scalar2=None,
                        op0=ALU.mult)
                    nc.vector.tensor_sub(dot[:], dot[:], mterm[:])
                    nc.vector.tensor_mul(dot[:], dot[:],
                                         ln2rstd[:, t : t + 1])
                    nc.vector.tensor_add(glog[:, e_ : e_ + 1], dot[:],
                                         bias4b[:, e_ : e_ + 1])
                m1 = gtp.tile([128, 1], F32, tag="m1", bufs=2, name="m1")
                nc.vector.reduce_max(m1[:], glog[:], axis=AX.X)
                eq1 = gtp.tile([128, E], F32, tag="eq1", bufs=2, name="eq1")
                nc.vector.tensor_scalar(out=eq1[:], in0=glog[:], scalar1=m1[:],
                                        scalar2=None, op0=ALU.is_equal)
                big = gtp.tile([128, E], F32, tag="big", bufs=2, name="big")
                nc.vector.tensor_scalar(out=big[:], in0=eq1[:], scalar1=1e30,
                                        scalar2=None, op0=ALU.mult)
                msk = gtp.tile([128, E], F32, tag="msk", bufs=2, name="msk")
                nc.vector.tensor_sub(msk[:], glog[:], big[:])
                m2 = gtp.tile([128, 1], F32, tag="m2", bufs=2, name="m2")
                nc.vector.reduce_max(m2[:], msk[:], axis=AX.X)
                eq2 = gtp.tile([128, E], F32, tag="eq2", bufs=2, name="eq2")
                nc.vector.tensor_scalar(out=eq2[:], in0=msk[:], scalar1=m2[:],
                                        scalar2=None, op0=ALU.is_equal)
                nm1 = gtp.tile([128, 1], F32, tag="nm1", bufs=2, name="nm1")
                nc.vector.tensor_scalar(out=nm1[:], in0=m1[:], scalar1=-1.0,
                                        scalar2=None, op0=ALU.mult)
                dx = gtp.tile([128, 1], F32, tag="dx", bufs=2, name="dx")
                nc.scalar.activation(dx[:], m2[:], AF.Exp, bias=nm1[:], scale=1.0)
                sden = gtp.tile([128, 1], F32, tag="sden", bufs=2, name="sden")
                nc.vector.tensor_scalar(out=sden[:], in0=dx[:], scalar1=1.0,
                                        scalar2=None, op0=ALU.add)
                w1s = gtp.tile([128, 1], F32, tag="w1s", bufs=2, name="w1s")
                nc.vector.reciprocal(w1s[:], sden[:])
                w2s = gtp.tile([128, 1], F32, tag="w2s", bufs=2, name="w2s")
                nc.vector.tensor_mul(w2s[:], dx[:], w1s[:])
                ga = gtp.tile([128, E], F32, tag="ga", bufs=2, name="ga")
                nc.vector.tensor_scalar(out=ga[:], in0=eq1[:], scalar1=w1s[:],
                                        scalar2=None, op0=ALU.mult)
                gb = gtp.tile([128, E], F32, tag="gb", bufs=2, name="gb")
                nc.vector.tensor_scalar(out=gb[:], in0=eq2[:], scalar1=w2s[:],
                                        scalar2=None, op0=ALU.mult)
                nc.vector.tensor_add(gates[t][:], ga[:], gb[:])
                nc.vector.tensor_scalar(out=gates32[t][:], in0=gates[t][:],
                                        scalar1=1.0 / WSCALE, scalar2=None,
                                        op0=ALU.mult)
            for e in (range(E) if stop_stage == "full" else range(0)):
                w2_sb = []
                for fp in range(FC // 2):
                    wt_ = w2p.tile([128, 2, D], FP8, tag="w2f", bufs=13,
                                   name="w2f")
                    nc.sync.dma_start(wt_[:], w2_d[e, fp].rearrange(
                        "p (i d) -> p i d", i=2))
                    w2_sb.append(wt_)

                hidT = []
                for fcg in range(DC):
                    w1g = w1p.tile([128, 3, 2, 512], FP8, tag="w1g", bufs=2,
                                   name="w1g")
                    nc.sync.dma_start(w1g[:], w1_d[e, fcg].rearrange(
                        "p (c i f) -> p c i f", c=3, i=2))
                    for fcm in range(4):
                        fc = fcg * 4 + fcm
                        ps = ps_mm.tile([128, 512], F32, tag="mm", name="ps_h")
                        for c in range(3):
                            nc.tensor.matmul(
                                ps[:],
                                w1g[:, c, :, fcm * 128 : (fcm + 1) * 128],
                                h2f8[:, 2 * c : 2 * c + 2, :],
                                start=(c == 0), stop=(c == 2),
                                perf_mode=mybir.MatmulPerfMode.DoubleRow)
                        if fc % 2 == 0:
                            hpair = hidp.tile([128, 2, 512], FP8, tag="hid",
                                              bufs=14, name="hid")
                            hidT.append(hpair)
                        nc.scalar.activation(
                            hidT[fc // 2][:, fc % 2, :], ps[:],
                            AF.Gelu_apprx_tanh,
                            bias=b1t_sb[:, e * FC + fc : e * FC + fc + 1],
                            scale=1.0 / WSCALE)

                for t in range(QT):
                    pya = ps_mm.tile([128, 512], F32, tag="mm", name="ps_ya")
                    pyb = ps_cx.tile([128, 512], F32, tag="cx", name="ps_yb")
                    for fp in range(FC // 2):
                        nc.tensor.matmul(
                            pya[:, :384],
                            hidT[fp][:, :, t * 128 : (t + 1) * 128],
                            w2_sb[fp][:, :, 0:384],
                            start=(fp == 0), stop=(fp == FC // 2 - 1),
                            perf_mode=mybir.MatmulPerfMode.DoubleRow)
                        nc.tensor.matmul(
                            pyb[:, :384],
                            hidT[fp][:, :, t * 128 : (t + 1) * 128],
                            w2_sb[fp][:, :, 384:768],
                            start=(fp == 0), stop=(fp == FC // 2 - 1),
                            perf_mode=mybir.MatmulPerfMode.DoubleRow)
                    for hf, py in ((0, pya), (1, pyb)):
                        ys = junkp.tile([128, 384], F32, tag="ys", bufs=2,
                                        name="ys")
                        nc.scalar.activation(ys[:], py[:, :384], AF.Identity,
                                             bias=0.0,
                                             scale=gates32[t][:, e : e + 1])
                        nc.vector.tensor_add(
                            xres[t][:, hf * 384 : (hf + 1) * 384],
                            xres[t][:, hf * 384 : (hf + 1) * 384], ys[:])
                    if e == E - 1:
                        nc.sync.dma_start(out_d[t * 128 : (t + 1) * 128, :],
                                          xres[t][:])
            gtp_cm.__exit__(None, None, None)

    nc.compile()
    return nc


def prepare_inputs(inputs):
    x = np.asarray(inputs["x"], np.float32)
    ln1_g = np.asarray(inputs["ln1_g"], np.float32)
    ln1_b = np.asarray(inputs["ln1_b"], np.float32)
    Wq = np.asarray(inputs["Wq"], np.float32)
    Wk = np.asarray(inputs["Wk"], np.float32)
    Wv = np.asarray(inputs["Wv"], np.float32)
    Wo = np.asarray(inputs["Wo"], np.float32)
    Wt = np.asarray(inputs["Wt"], np.float32)
    bt = np.asarray(inputs["bt"], np.float32)
    ln2_g = np.asarray(inputs["ln2_g"], np.float32)
    ln2_b = np.asarray(inputs["ln2_b"], np.float32)
    Wg = np.asarray(inputs["Wg"], np.float32)
    W1 = np.asarray(inputs["W1"], np.float32)
    b1 = np.asarray(inputs["b1"], np.float32)
    W2 = np.asarray(inputs["W2"], np.float32)
    b2 = np.asarray(inputs["b2"], np.float32)

    bv = ln1_b @ Wv
    if np.any(b2) or np.any(bv):
        raise NotImplementedError("nonzero b2 / ln1_b@Wv path not implemented")

    wq = (ln1_g[:, None] * Wq).astype(BF16NP)
    wk = (ln1_g[:, None] * Wk).astype(BF16NP)
    wv = (ln1_g[:, None] * Wv).astype(BF16NP)
    wo = Wo.astype(BF16NP)
    bq = ln1_b @ Wq
    bk = ln1_b @ Wk
    bqk = np.concatenate([bq.reshape(DC, 128).T, bk.reshape(DC, 128).T],
                         axis=1).astype(np.float32)
    has_qkbias = bool(np.any(bqk))

    wg = (ln2_g[:, None] * Wg).astype(BF16NP)
    wg_r = np.ascontiguousarray(
        wg.reshape(DC, 128, E).transpose(1, 0, 2).reshape(128, DC * E))
    bg = (ln2_b @ Wg).reshape(1, E).astype(np.float32)
    wg32 = (ln2_g[:, None] * Wg).astype(np.float32)
    wg_bcast = np.ascontiguousarray(np.broadcast_to(
        wg32.T.reshape(1, E * D), (128, E * D)))
    swg = wg32.sum(axis=0)

    # fp8 DoubleRow layout for W1: [e, fcg, p, (c i fgrp)] where the
    # contraction index is d = c*256 + i*128 + p.
    w1 = ((ln2_g[None, :, None] * W1) * WSCALE).astype(FP8NP)
    w1r = np.ascontiguousarray(
        w1.reshape(E, 3, 2, 128, DC, 512).transpose(0, 4, 3, 1, 2, 5)
        .reshape(E, DC, 128, 3 * 2 * 512))
    b1_tot = (b1 + np.einsum("d,edf->ef", ln2_b, W1)).astype(np.float32)
    b1t = np.ascontiguousarray(
        b1_tot.reshape(E, FC, 128).transpose(2, 0, 1).reshape(128, E * FC))
    # fp8 DoubleRow layout for W2: [e, fp, p, (i d)], contraction index
    # f = fp*256 + i*128 + p.
    w2 = (W2 * WSCALE).astype(FP8NP)
    w2r = np.ascontiguousarray(
        w2.reshape(E, FC // 2, 2, 128, D).transpose(0, 1, 3, 2, 4)
        .reshape(E, FC // 2, 128, 2 * D))
    wt_t = np.ascontiguousarray(Wt.T).astype(BF16NP).reshape(1, 2 * D)

    meta = {"bt0": float(bt[0]), "bt1": float(bt[1]),
            "has_qkbias": has_qkbias, "swg": [float(v) for v in swg]}
    shared = {
        "wq": wq, "wk": wk, "wv": wv, "wo": wo, "wt_t": wt_t,
        "wg_r": wg_r, "bg": bg, "w1r": w1r, "w2r": w2r, "b1t": b1t,
        "bqk": bqk, "wg_bcast": wg_bcast,
    }
    in_maps = []
    for i in range(N_CORES):
        b, half = i // 2, i % 2
        m = dict(shared)
        m["x_kv"] = np.ascontiguousarray(x[b])
        m["x_q"] = np.ascontiguousarray(x[b, half * NQ : (half + 1) * NQ])
        in_maps.append(m)
    return meta, in_maps


def kernel(**inputs):
    meta, in_maps = prepare_inputs(inputs)
    key = ("v1", meta["has_qkbias"], meta["bt0"], meta["bt1"], tuple(meta["swg"]))
    if key not in _CACHED:
        _CACHED[key] = build_program(meta)
    nc = _CACHED[key]

    res = run_bass_kernel_spmd(nc, in_maps, list(range(N_CORES)),
                               trace=bool(inputs.get("_trace", False)))
    out = np.empty((B, S, D), np.float32)
    for i in range(N_CORES):
        b, half = i // 2, i % 2
        out[b, half * NQ : (half + 1) * NQ] = res.results[i]["out"]
    if inputs.get("_want_time", False):
        return out, res
    return out


# revision 46
# speedup vs baseline: 1.0741x; 1.0261x over previous
"""Trainium2 Bass kernel for a transformer block with attention + top-2-of-4 MoE.

Problem (B=4, S=1024, D=768, H=12, E=4, DF=3072, TOPK=2):
  pooled task-routing bias -> pre-norm MHA with residual -> pre-norm top-2 MoE
  with routing bias, residual.

Sharding: 8 cores, token-parallel. Core i handles batch b=i//2, sequence half
i%2 (512 query tokens). Each core gets the full 1024-token sequence of its
batch (x_kv) to compute K/V and the pooled routing bias locally; no
collectives are needed. All weights are replicated, pre-cast to bf16 on the
host, with LayerNorm gammas/betas folded into the consuming weight matrices.

Attention runs in bf16 on the TensorEngine with fp32 PSUM accumulation;
LayerNorm statistics, softmax denominators and residuals stay fp32. Softmax
runs without max-subtraction (logits are O(1) for this problem's weight
scale), which lets scores be computed directly in [key, query] layout so no
attention-probability transposes are needed: the denominator comes from an
appended ones-column in the value tensor and 1/den = exp(-ln(den)) is applied
when evicting per-head context (vector.reciprocal on a [1, N] row is ~8x-slow
single-lane DVE work). The two heads of each pair occupy PE row groups 0 and
64 so their K=64 score matmuls run concurrently (a lone K=64 matmul streams
at half rate).

The MoE expert FFNs run as fp8e4m3 DoubleRow matmuls (weights host-quantized
with a x32 scale folded back out via the gelu input scale and a pre-scaled
gates vector), ~1.4x the bf16 TensorEngine throughput. Gate logits are
computed in full fp32 on the VectorEngine (overlapping expert matmuls) since
bf16 logit noise flips the top-2 selection on knife-edge tokens.
"""

import contextlib

import numpy as np
import ml_dtypes

import concourse.bass as bass
import concourse.bacc as bacc
import concourse.mybir as mybir
import concourse.tile as tile
from concourse import bass_isa, masks
from concourse.bass_utils import run_bass_kernel_spmd

B, S, D, H, E, DF = 4, 1024, 768, 12, 4, 3072
HD = D // H          # 64
NQ = S // 2          # 512 tokens owned per core
NKV = S
N_CORES = 8
DC = D // 128        # 6 d-chunks
FC = DF // 128       # 24 f-chunks
QT = NQ // 128       # 4 query token tiles
KT = NKV // 128      # 8 kv token tiles
EPS = 1e-5

F32 = mybir.dt.float32
BF16 = mybir.dt.bfloat16
AF = mybir.ActivationFunctionType
ALU = mybir.AluOpType
AX = mybir.AxisListType
BF16NP = ml_dtypes.bfloat16
FP8 = mybir.dt.float8e4
FP8NP = ml_dtypes.float8_e4m3
WSCALE = 32.0

_CACHED = {}


def _layernorm_tiles(nc, small, junkp, x_tiles, n_tiles, xn_pool, name,
                     eps_ap=None):
    """LN over [128, D] f32 tiles -> bf16 normalized tiles (gamma/beta folded
    into downstream weights on the host). Returns list of bf16 [128, D] tiles."""
    stats_s = small.tile([128, n_tiles], F32, tag=f"{name}_s", name=f"{name}_s")
    stats_q = small.tile([128, n_tiles], F32, tag=f"{name}_q", name=f"{name}_q")
    for t in range(n_tiles):
        nc.vector.reduce_sum(stats_s[:, t : t + 1], x_tiles[t][:], axis=AX.X)
        j = junkp.tile([128, D], BF16, tag="junk", bufs=2, name="junk")
        nc.scalar.activation(j[:], x_tiles[t][:], AF.Square,
                             accum_out=stats_q[:, t : t + 1])
    mu = small.tile([128, n_tiles], F32, tag=f"{name}_mu", name=f"{name}_mu")
    var = small.tile([128, n_tiles], F32, tag=f"{name}_var", name=f"{name}_var")
    nc.vector.tensor_scalar(out=mu[:], in0=stats_s[:], scalar1=1.0 / D,
                            scalar2=None, op0=ALU.mult)
    nc.vector.tensor_scalar(out=var[:], in0=stats_q[:], scalar1=1.0 / D,
                            scalar2=None, op0=ALU.mult)
    mu2 = small.tile([128, n_tiles], F32, tag=f"{name}_mu2", name=f"{name}_mu2")
    nc.vector.tensor_mul(mu2[:], mu[:], mu[:])
    nc.vector.tensor_sub(var[:], var[:], mu2[:])
    lnv = small.tile([128, n_tiles], F32, tag=f"{name}_lnv", name=f"{name}_lnv")
    nc.scalar.activation(lnv[:], var[:], AF.Ln, bias=eps_ap[:], scale=1.0)
    rstd = small.tile([128, n_tiles], F32, tag=f"{name}_rstd", name=f"{name}_rstd")
    nc.scalar.activation(rstd[:], lnv[:], AF.Exp, scale=-0.5)
    nmr = small.tile([128, n_tiles], F32, tag=f"{name}_nmr", name=f"{name}_nmr")
    nc.vector.tensor_mul(nmr[:], mu[:], rstd[:])
    nc.vector.tensor_scalar(out=nmr[:], in0=nmr[:], scalar1=-1.0, scalar2=None,
                            op0=ALU.mult)
    xn_tiles = []
    for t in range(n_tiles):
        xn = xn_pool.tile([128, D], BF16, tag=f"{name}_xn", bufs=n_tiles,
                          name=f"{name}_xn")
        nc.scalar.activation(xn[:], x_tiles[t][:], AF.Identity,
                             bias=nmr[:, t : t + 1], scale=rstd[:, t : t + 1])
        xn_tiles.append(xn)
    return xn_tiles, mu, rstd


def _transpose_to(nc, ps_tr, ident, xn_tiles, dst_pool, n_tok_tiles, name,
                  dst_bufs=6):
    """Transpose token-major bf16 tiles [128tok, D] into d-major tiles
    [128d, n_tok_tiles*128]. Returns list of DC tiles."""
    dst = []
    for d in range(DC):
        dt_ = dst_pool.tile([128, n_tok_tiles * 128], BF16, tag=f"{name}_T",
                            bufs=dst_bufs, name=f"{name}_T")
        ps = ps_tr.tile([128, 1024], BF16, tag="big", bufs=2, name="tr_ps")
        for t in range(n_tok_tiles):
            nc.tensor.transpose(ps[:, t * 128 : (t + 1) * 128],
                                xn_tiles[t][:, d * 128 : (d + 1) * 128], ident[:])
        nc.vector.tensor_copy(dt_[:], ps[:, : n_tok_tiles * 128])
        dst.append(dt_)
    return dst


def build_program(meta, stop_stage='full'):
    nc = bacc.Bacc("TRN2", target_bir_lowering=False, debug=False,
                   num_devices=N_CORES)

    # ---- DRAM parameters --------------------------------------------------
    xq_d = nc.declare_dram_parameter("x_q", [NQ, D], F32, isOutput=False)
    xkv_d = nc.declare_dram_parameter("x_kv", [NKV, D], F32, isOutput=False)
    wq_d = nc.declare_dram_parameter("wq", [D, D], BF16, isOutput=False)
    wk_d = nc.declare_dram_parameter("wk", [D, D], BF16, isOutput=False)
    wv_d = nc.declare_dram_parameter("wv", [D, D], BF16, isOutput=False)
    wo_d = nc.declare_dram_parameter("wo", [D, D], BF16, isOutput=False)
    wtt_d = nc.declare_dram_parameter("wt_t", [1, 2 * D], BF16, isOutput=False)
    wg_d = nc.declare_dram_parameter("wg_r", [128, DC * E], BF16, isOutput=False)
    bg_d = nc.declare_dram_parameter("bg", [1, E], F32, isOutput=False)
    wgb_d = nc.declare_dram_parameter("wg_bcast", [128, E * D], F32, isOutput=False)
    w1_d = nc.declare_dram_parameter("w1r", [E, DC, 128, 3 * 2 * 512], FP8, isOutput=False)
    w2_d = nc.declare_dram_parameter("w2r", [E, FC // 2, 128, 2 * D], FP8, isOutput=False)
    b1_d = nc.declare_dram_parameter("b1t", [128, E * FC], F32, isOutput=False)
    bqk_d = nc.declare_dram_parameter("bqk", [128, 2 * DC], F32, isOutput=False)
    out_d = nc.declare_dram_parameter("out", [NQ, D], F32, isOutput=True)

    has_qkbias = meta["has_qkbias"]
    bt0, bt1 = meta["bt0"], meta["bt1"]

    with tile.TileContext(nc) as tc, contextlib.ExitStack() as ctx:
        glob = ctx.enter_context(tc.tile_pool(name="glob", bufs=1))
        small = ctx.enter_context(tc.tile_pool(name="small", bufs=1))
        junkp = ctx.enter_context(tc.tile_pool(name="junkp", bufs=2))

        ps_tr = ctx.enter_context(tc.tile_pool(name="ps_tr", bufs=2, space="PSUM"))
        ps_mm = ctx.enter_context(tc.tile_pool(name="ps_mm", bufs=2, space="PSUM"))
        ps_cx = ctx.enter_context(tc.tile_pool(name="ps_cx", bufs=2, space="PSUM"))
        ps_s = ps_tr  # scores share the 2x [128,1024]-sized slots with transposes

        ident = glob.tile([128, 128], BF16, tag="ident", name="ident")
        masks.make_identity(nc, ident[:])
        eps_sb = glob.tile([128, 1], F32, tag="eps", name="eps_sb")
        nc.vector.memset(eps_sb[:], EPS)

        xq_sb = []
        for t in range(QT):
            xt = glob.tile([128, D], F32, tag="xq", bufs=QT, name="xq")
            nc.sync.dma_start(xt[:], xq_d[t * 128 : (t + 1) * 128, :])
            xq_sb.append(xt)

        # MoE weight streaming pools (w1 global so expert 0 prefetches early)
        w1p = ctx.enter_context(tc.tile_pool(name="w1p", bufs=3))
        b1t_sb = glob.tile([128, E * FC], F32, tag="b1t", name="b1t")
        nc.sync.dma_start(b1t_sb[:], b1_d[:])
        wg_sb = glob.tile([128, DC, E], BF16, tag="wg", name="wg")
        nc.sync.dma_start(wg_sb[:], wg_d[:].rearrange("p (d e) -> p d e", e=E))
        bg_sb = glob.tile([1, E], F32, tag="bg", name="bg")
        nc.sync.dma_start(bg_sb[:], bg_d[:])

        xres = [glob.tile([128, D], F32, tag="xres", bufs=QT, name="xres")
                for _ in range(QT)]
        gates = [glob.tile([128, E], F32, tag="gates", bufs=QT, name="gates")
                 for _ in range(QT)]
        bias4 = glob.tile([1, E], F32, tag="bias4", name="bias4")
        h2T = []

        # ---- attention phase ---------------------------------------------
        with (
            tc.tile_pool(name="attn", bufs=1) as attn,
            tc.tile_pool(name="xnp", bufs=8) as xnp,
        ):
            wsub_cm = tc.tile_pool(name="wsub", bufs=1)
            wsub = wsub_cm.__enter__()
            xkv_sb = []
            for t in range(KT):
                xt = attn.tile([128, D], F32, tag="xkv", bufs=KT, name="xkv")
                nc.sync.dma_start(xt[:], xkv_d[t * 128 : (t + 1) * 128, :])
                xkv_sb.append(xt)
            wq_sb, wk_sb, wv_sb, wo_sb = [], [], [], []
            for (wd, lst, tg) in ((wq_d, wq_sb, "wq"), (wk_d, wk_sb, "wk"),
                                  (wv_d, wv_sb, "wv")):
                for d in range(DC):
                    wt_ = wsub.tile([128, D], BF16, tag=tg, bufs=DC, name=tg)
                    nc.sync.dma_start(wt_[:], wd[d * 128 : (d + 1) * 128, :])
                    lst.append(wt_)
            for d in range(DC):
                wt2_ = attn.tile([128, D], BF16, tag="wo", bufs=DC, name="wo")
                nc.sync.dma_start(wt2_[:], wo_d[d * 128 : (d + 1) * 128, :])
                wo_sb.append(wt2_)
            wtt_sb = attn.tile([1, 2, D], BF16, tag="wtt", name="wtt")
            nc.sync.dma_start(wtt_sb[:], wtt_d[:].rearrange("p (j d) -> p j d", j=2))
            bqk_sb = attn.tile([128, 2 * DC], F32, tag="bqk", name="bqk")
            nc.sync.dma_start(bqk_sb[:], bqk_d[:])

            # pooled routing bias: pooled = mean_t x_kv; tl = pooled @ Wt + bt
            # (partition sum on GpSimd to keep it off the TensorEngine)
            pooled = attn.tile([1, D], F32, tag="pooled", name="pooled")
            psum_x = junkp.tile([128, D], BF16, tag="junk", bufs=2,
                                name="junk")
            nc.vector.tensor_add(psum_x[:], xkv_sb[0][:], xkv_sb[1][:])
            for t in range(2, KT):
                nc.vector.tensor_add(psum_x[:], psum_x[:], xkv_sb[t][:])
            pall = junkp.tile([128, D], BF16, tag="junk", bufs=2, name="junk")
            nc.gpsimd.partition_all_reduce(pall[:], psum_x[:], channels=128,
                                           reduce_op=bass_isa.ReduceOp.add)
            nc.vector.tensor_scalar(out=pooled[:], in0=pall[0:1, :],
                                    scalar1=1.0 / NKV, scalar2=None,
                                    op0=ALU.mult)
            tl = attn.tile([1, 2], F32, tag="tl", name="tl")
            for j, btj in ((0, bt0), (1, bt1)):
                jrow = junkp.tile([128, D], BF16, tag="junk", bufs=2, name="junk")
                nc.vector.tensor_mul(jrow[0:1, :], pooled[:], wtt_sb[:, j, :])
                nc.vector.reduce_sum(tl[:, j : j + 1], jrow[0:1, :], axis=AX.X)
                nc.vector.tensor_scalar(out=tl[:, j : j + 1],
                                        in0=tl[:, j : j + 1],
                                        scalar1=float(btj), scalar2=None,
                                        op0=ALU.add)
            for e_, j_ in ((0, 0), (1, 1), (2, 0), (3, 1)):  # DOMAIN_MAP
                nc.vector.tensor_copy(bias4[:, e_ : e_ + 1], tl[:, j_ : j_ + 1])
            nc.vector.tensor_add(bias4[:], bias4[:], bg_sb[:])
            bias4b = glob.tile([128, E], F32, tag="bias4b", name="bias4b")
            nc.gpsimd.partition_broadcast(bias4b[:], bias4[:])

            # LN1 -> transposed bf16 activations
            xn_kv, _, _ = _layernorm_tiles(nc, small, junkp, xkv_sb, KT, xnp, "ln1kv", eps_sb)
            xn_q, _, _ = _layernorm_tiles(nc, small, junkp, xq_sb, QT, xnp, "ln1q", eps_sb)
            hkvT = _transpose_to(nc, ps_tr, ident, xn_kv, attn, KT, "hkv")
            hqT = _transpose_to(nc, ps_tr, ident, xn_q, attn, QT, "hq")

            # Q/K projections -> d-major qT [D, NQ], kT [D, NKV]
            qT = [attn.tile([128, NQ], BF16, tag="qT", bufs=DC, name="qT")
                  for _ in range(DC)]
            kT = [attn.tile([128, NKV], BF16, tag="kT", bufs=DC, name="kT")
                  for _ in range(DC)]
            for m in range(DC):
                ps = ps_mm.tile([128, 512], F32, tag="mm", name="ps_q")
                for d in range(DC):
                    nc.tensor.matmul(ps[:], wq_sb[d][:, m * 128 : (m + 1) * 128],
                                     hqT[d][:], start=(d == 0), stop=(d == DC - 1))
                if has_qkbias:
                    nc.scalar.activation(qT[m][:], ps[:], AF.Identity,
                                         bias=bqk_sb[:, m : m + 1], scale=1.0)
                else:
                    nc.vector.tensor_copy(qT[m][:], ps[:])
                for hf in range(2):
                    ps2 = ps_mm.tile([128, 512], F32, tag="mm", name="ps_k")
                    for d in range(DC):
                        nc.tensor.matmul(
                            ps2[:], wk_sb[d][:, m * 128 : (m + 1) * 128],
                            hkvT[d][:, hf * 512 : (hf + 1) * 512],
                            start=(d == 0), stop=(d == DC - 1))
                    if has_qkbias:
                        nc.scalar.activation(kT[m][:, hf * 512 : (hf + 1) * 512],
                                             ps2[:], AF.Identity,
                                             bias=bqk_sb[:, DC + m : DC + m + 1],
                                             scale=1.0)
                    else:
                        nc.vector.tensor_copy(kT[m][:, hf * 512 : (hf + 1) * 512],
                                              ps2[:])

            # V projection (token-major) into v_aug [128, H, HD+1], ones col
            v_aug = [attn.tile([128, H, HD + 1], BF16, tag="vaug", bufs=KT,
                               name="vaug") for _ in range(KT)]
            for t in range(KT):
                for hf in range(2):
                    ps = ps_mm.tile([128, 512], F32, tag="mm", name="ps_v")
                    for d in range(DC):
                        nc.tensor.matmul(
                            ps[:, :384],
                            hkvT[d][:, t * 128 : (t + 1) * 128],
                            wv_sb[d][:, hf * 384 : (hf + 1) * 384],
                            start=(d == 0), stop=(d == DC - 1))
                    nc.vector.tensor_copy(
                        v_aug[t][:, hf * 6 : (hf + 1) * 6, :HD],
                        ps[:, :384].rearrange("p (h x) -> p h x", h=6))
                nc.vector.memset(v_aug[t][:, :, HD : HD + 1], 1.0)
            wsub_cm.__exit__(None, None, None)
            pp_cm = tc.tile_pool(name="pp", bufs=6)
            pp = pp_cm.__enter__()

            if stop_stage == "qkv":
                for t in range(QT):
                    nc.sync.dma_start(out_d[t * 128 : (t + 1) * 128, :],
                                      xq_sb[t][:])
            # head-pair attention: the two heads of a pair live in row
            # groups 0 and 64 of the same qT/kT tile, so their K=64 score
            # matmuls execute concurrently on disjoint PE row groups.
            ctxT = [attn.tile([128, NQ], BF16, tag="ctxT", bufs=DC, name="ctxT")
                    for _ in range(DC)]
            pairs = range(H // 2) if stop_stage != "qkv" else range(0)
            def _emit_scores(j):
                p_sb = []
                for c in range(KT):
                    pss = ps_s.tile([128, 1024], F32, tag="big", bufs=2,
                                    name="ps_s")
                    nc.tensor.matmul(
                        pss[:, 0:512],
                        kT[j][0:HD, c * 128 : (c + 1) * 128],
                        qT[j][0:HD, :], start=True, stop=True)
                    nc.tensor.matmul(
                        pss[:, 512:1024],
                        kT[j][HD:128, c * 128 : (c + 1) * 128],
                        qT[j][HD:128, :], start=True, stop=True)
                    pc = pp.tile([128, 1024], BF16, tag="p", bufs=9, name="p")
                    nc.scalar.activation(pc[:], pss[:], AF.Exp,
                                         scale=float(1.0 / np.sqrt(HD)))
                    p_sb.append(pc)
                return p_sb

            def _emit_scores_interleaved(j, prev_j, prev_p, prev_cx):
                # chunk-interleave pair j's score matmuls with pair prev_j's
                # AV matmuls so PE work is available while ScalarE streams
                # the exps for pair j.
                p_sb = []
                pcx0, pcx1 = prev_cx
                for c in range(KT):
                    nc.tensor.matmul(pcx0[:], v_aug[c][:, 2 * prev_j, :],
                                     prev_p[c][:, 0:512],
                                     start=(c == 0), stop=(c == KT - 1))
                    nc.tensor.matmul(pcx1[:], v_aug[c][:, 2 * prev_j + 1, :],
                                     prev_p[c][:, 512:1024],
                                     start=(c == 0), stop=(c == KT - 1))
                    pss = ps_s.tile([128, 1024], F32, tag="big", bufs=2,
                                    name="ps_s")
                    nc.tensor.matmul(
                        pss[:, 0:512],
                        kT[j][0:HD, c * 128 : (c + 1) * 128],
                        qT[j][0:HD, :], start=True, stop=True)
                    nc.tensor.matmul(
                        pss[:, 512:1024],
                        kT[j][HD:128, c * 128 : (c + 1) * 128],
                        qT[j][HD:128, :], start=True, stop=True)
                    pc = pp.tile([128, 1024], BF16, tag="p", bufs=9, name="p")
                    nc.scalar.activation(pc[:], pss[:], AF.Exp,
                                         scale=float(1.0 / np.sqrt(HD)))
                    p_sb.append(pc)
                return p_sb

            def _emit_av(j, p_sb):
                pcx0 = ps_cx.tile([HD + 1, 512], F32, tag="cx", name="ps_cx")
                pcx1 = ps_mm.tile([HD + 1, 512], F32, tag="mm", name="ps_cx1")
                for c in range(KT):
                    nc.tensor.matmul(pcx0[:], v_aug[c][:, 2 * j, :],
                                     p_sb[c][:, 0:512],
                                     start=(c == 0), stop=(c == KT - 1))
                    nc.tensor.matmul(pcx1[:], v_aug[c][:, 2 * j + 1, :],
                                     p_sb[c][:, 512:1024],
                                     start=(c == 0), stop=(c == KT - 1))
                return pcx0, pcx1

            def _emit_norm(j, pcx0, pcx1):
                den2 = attn.tile([1, 1024], BF16, tag="den2", bufs=1,
                                 name="den2")
                cu = attn.tile([64, 512], BF16, tag="cu", bufs=2, name="cu")
                nc.vector.tensor_copy(ctxT[j][0:HD, :], pcx0[:HD, :])
                nc.vector.tensor_copy(den2[:, 0:512], pcx0[HD : HD + 1, :])
                nc.vector.tensor_copy(cu[:], pcx1[:HD, :])
                nc.vector.tensor_copy(den2[:, 512:1024], pcx1[HD : HD + 1, :])
                lnd = attn.tile([1, 1024], F32, tag="lnd", bufs=1, name="lnd")
                nc.scalar.activation(lnd[:], den2[:], AF.Ln)
                rdenb2 = attn.tile([1, 1024], BF16, tag="rdenb2", bufs=1,
                                   name="rdenb2")
                nc.scalar.activation(rdenb2[:], lnd[:], AF.Exp, scale=-1.0)
                bcs0 = attn.tile([64, 512], BF16, tag="bcs", bufs=2,
                                 name="bcs")
                nc.gpsimd.partition_broadcast(bcs0[:], rdenb2[:, 0:512])
                nc.vector.tensor_mul(ctxT[j][0:HD, :], ctxT[j][0:HD, :],
                                     bcs0[:])
                bcs1 = attn.tile([64, 512], BF16, tag="bcs", bufs=2,
                                 name="bcs")
                nc.gpsimd.partition_broadcast(bcs1[:], rdenb2[:, 512:1024])
                nc.vector.tensor_mul(ctxT[j][HD:128, :], cu[:], bcs1[:])

            prev = None
            for j in pairs:
                if prev is None:
                    p_sb = _emit_scores(j)
                else:
                    pj, pp_sb = prev
                    pcx = _emit_av_alloc = None
                    pcx0 = ps_cx.tile([HD + 1, 512], F32, tag="cx",
                                      name="ps_cx")
                    pcx1 = ps_mm.tile([HD + 1, 512], F32, tag="mm",
                                      name="ps_cx1")
                    p_sb = _emit_scores_interleaved(j, pj, pp_sb,
                                                    (pcx0, pcx1))
                    _emit_norm(pj, pcx0, pcx1)
                prev = (j, p_sb)
            if prev is not None:
                pj, pp_sb = prev
                pcx0, pcx1 = _emit_av(pj, pp_sb)
                _emit_norm(pj, pcx0, pcx1)

            # output projection + residual -> xres (f32)
            for t in (range(QT) if stop_stage not in ("qkv", "heads") else range(0)):
                for hf in range(2):
                    ps = ps_mm.tile([128, 512], F32, tag="mm", name="ps_o")
                    for d in range(DC):
                        nc.tensor.matmul(
                            ps[:, :384],
                            ctxT[d][:, t * 128 : (t + 1) * 128],
                            wo_sb[d][:, hf * 384 : (hf + 1) * 384],
                            start=(d == 0), stop=(d == DC - 1))
                    nc.vector.tensor_add(xres[t][:, hf * 384 : (hf + 1) * 384],
                                         ps[:, :384],
                                         xq_sb[t][:, hf * 384 : (hf + 1) * 384])

            if stop_stage == "heads":
                for t in range(QT):
                    nc.sync.dma_start(out_d[t * 128 : (t + 1) * 128, :],
                                      xq_sb[t][:])
            if stop_stage == "attn":
                for t in range(QT):
                    nc.sync.dma_start(out_d[t * 128 : (t + 1) * 128, :],
                                      xres[t][:])
            # LN2 + transpose into h2T (kept in glob for MoE phase)
            do_rest = stop_stage in ("full", "gates")
            if do_rest:
                xn2, ln2mu, ln2rstd = _layernorm_tiles(
                    nc, small, junkp, xres, QT, xnp, "ln2", eps_sb)
                h2f8 = glob.tile([128, DC, 512], FP8, tag="h2f8", name="h2f8")
                for d in range(DC):
                    dt_ = glob.tile([128, 512], BF16, tag="h2_T", bufs=DC,
                                    name="h2_T")
                    ps = ps_tr.tile([128, 1024], BF16, tag="big", bufs=2,
                                    name="tr_ps")
                    for t in range(QT):
                        nc.tensor.transpose(ps[:, t * 128 : (t + 1) * 128],
                                            xn2[t][:, d * 128 : (d + 1) * 128],
                                            ident[:])
                    nc.vector.tensor_copy(dt_[:], ps[:, :512])
                    nc.scalar.copy(h2f8[:, d, :], ps[:, :512])
                    h2T.append(dt_)

            # gate logits + top-2 softmax gates
            pp_cm.__exit__(None, None, None)

        if stop_stage == "gates":
            for t in range(QT):
                nc.sync.dma_start(out_d[t * 128 : (t + 1) * 128, :], xres[t][:])
        # ---- MoE phase (fp8 DoubleRow matmuls) -----------------------------
        # W1/W2 are host-quantized to fp8e4 scaled by WSCALE; the 1/WSCALE is
        # folded into the gelu input scale (W1) and a pre-scaled gates vector
        # (W2). Contraction runs in K=256 DoubleRow chunks: both operands are
        # [128, 2, N] with the pair index selecting the two K-subchunks.
        with (
            tc.tile_pool(name="hidp", bufs=26) as hidp,
            tc.tile_pool(name="w2p", bufs=13) as w2p,
        ):
            gtp_cm = tc.tile_pool(name="gtp", bufs=1)
            gtp = gtp_cm.__enter__()
            gates32 = [glob.tile([128, E], F32, tag="gates32", bufs=QT,
                                 name="gates32") for _ in range(QT)]
            # Gate logits in full f32 on DVE (bf16 matmul noise flips the
            # top-2 selection on knife-edge tokens):
            #   logit[t, e] = rstd_t * (xres_t . wg_e - mu_t * sum(wg_e)) + bias_e
            if do_rest:
                wgb = [gtp.tile([128, D], F32, tag="wgb", bufs=E, name="wgb")
                       for _ in range(E)]
                for e_ in range(E):
                    nc.sync.dma_start(wgb[e_][:],
                                      wgb_d[:, e_ * D : (e_ + 1) * D])
            for t in (range(QT) if do_rest else range(0)):
                glog = gtp.tile([128, E], F32, tag="glog", bufs=2, name="glog")
                for e_ in range(E):
                    jr = gtp.tile([128, D], F32, tag="jr32", bufs=2,
                                 name="jr32")
                    nc.vector.tensor_mul(jr[:], xres[t][:], wgb[e_][:])
                    dot = gtp.tile([128, 1], F32, tag="gdot", bufs=2,
                                    name="gdot")
                    nc.vector.reduce_sum(dot[:], jr[:], axis=AX.X)
                    mterm = gtp.tile([128, 1], F32, tag="gmt", bufs=2,
                                      name="gmt")
                    nc.vector.tensor_scalar(
                        out=mterm[:], in0=ln2mu[:, t : t + 1],
                        scalar1=float(meta["swg"][e_]), scalar2=None,
                        op0=ALU.mult)
                    nc.vector.tensor_sub(dot[:], dot[:], mterm[:])
                    nc.vector.tensor_mul(dot[:], dot[:],
                                         ln2rstd[:, t : t + 1])
                    nc.vector.tensor_add(glog[:, e_ : e_ + 1], dot[:],
                                         bias4b[:, e_ : e_ + 1])
                m1 = gtp.tile([128, 1], F32, tag="m1", bufs=2, name="m1")
                nc.vector.reduce_max(m1[:], glog[:], axis=AX.X)
                eq1 = gtp.tile([128, E], F32, tag="eq1", bufs=2, name="eq1")
                nc.vector.tensor_scalar(out=eq1[:], in0=glog[:], scalar1=m1[:],
                                        scalar2=None, op0=ALU.is_equal)
                big = gtp.tile([128, E], F32, tag="big", bufs=2, name="big")
                nc.vector.tensor_scalar(out=big[:], in0=eq1[:], scalar1=1e30,
                                        scalar2=None, op0=ALU.mult)
                msk = gtp.tile([128, E], F32, tag="msk", bufs=2, name="msk")
                nc.vector.tensor_sub(msk[:], glog[:], big[:])
                m2 = gtp.tile([128, 1], F32, tag="m2", bufs=2, name="m2")
                nc.vector.reduce_max(m2[:], msk[:], axis=AX.X)
                eq2 = gtp.tile([128, E], F32, tag="eq2", bufs=2, name="eq2")
                nc.vector.tensor_scalar(out=eq2[:], in0=msk[:], scalar1=m2[:],
                                        scalar2=None, op0=ALU.is_equal)
                nm1 = gtp.tile([128, 1], F32, tag="nm1", bufs=2, name="nm1")
                nc.vector.tensor_scalar(out=nm1[:], in0=m1[:], scalar1=-1.0,
                                        scalar2=None, op0=ALU.mult)
                dx = gtp.tile([128, 1], F32, tag="dx", bufs=2, name="dx")
                nc.scalar.activation(dx[:], m2[:], AF.Exp, bias=nm1[:], scale=1.0)
                sden = gtp.tile([128, 1], F32, tag="sden", bufs=2, name="sden")
                nc.vector.tensor_scalar(out=sden[:], in0=dx[:], scalar1=1.0,
                                        scalar2=None, op0=ALU.add)
                w1s = gtp.tile([128, 1], F32, tag="w1s", bufs=2, name="w1s")
                nc.vector.reciprocal(w1s[:], sden[:])
                w2s = gtp.tile([128, 1], F32, tag="w2s", bufs=2, name="w2s")
                nc.vector.tensor_mul(w2s[:], dx[:], w1s[:])
                ga = gtp.tile([128, E], F32, tag="ga", bufs=2, name="ga")
                nc.vector.tensor_scalar(out=ga[:], in0=eq1[:], scalar1=w1s[:],
                                        scalar2=None, op0=ALU.mult)
                gb = gtp.tile([128, E], F32, tag="gb", bufs=2, name="gb")
                nc.vector.tensor_scalar(out=gb[:], in0=eq2[:], scalar1=w2s[:],
                                        scalar2=None, op0=ALU.mult)
                nc.vector.tensor_add(gates[t][:], ga[:], gb[:])
                nc.vector.tensor_scalar(out=gates32[t][:], in0=gates[t][:],
                                        scalar1=1.0 / WSCALE, scalar2=None,
                                        op0=ALU.mult)
            for e in (range(E) if stop_stage == "full" else range(0)):
                w2_sb = []
                for fp in range(FC // 2):
                    wt_ = w2p.tile([128, 2, D], FP8, tag="w2f", bufs=13,
                                   name="w2f")
                    nc.sync.dma_start(wt_[:], w2_d[e, fp].rearrange(
                        "p (i d) -> p i d", i=2))
                    w2_sb.append(wt_)

                hidT = []
                for fcg in range(DC):
                    w1g = w1p.tile([128, 3, 2, 512], FP8, tag="w1g", bufs=3,
                                   name="w1g")
                    nc.sync.dma_start(w1g[:], w1_d[e, fcg].rearrange(
                        "p (c i f) -> p c i f", c=3, i=2))
                    for fcm in range(4):
                        fc = fcg * 4 + fcm
                        ps = ps_mm.tile([128, 512], F32, tag="mm", name="ps_h")
                        for c in range(3):
                            nc.tensor.matmul(
                                ps[:],
                                w1g[:, c, :, fcm * 128 : (fcm + 1) * 128],
                                h2f8[:, 2 * c : 2 * c + 2, :],
                                start=(c == 0), stop=(c == 2),
                                perf_mode=mybir.MatmulPerfMode.DoubleRow)
                        if fc % 2 == 0:
                            hpair = hidp.tile([128, 2, 512], FP8, tag="hid",
                                              bufs=26, name="hid")
                            hidT.append(hpair)
                        nc.scalar.activation(
                            hidT[fc // 2][:, fc % 2, :], ps[:],
                            AF.Gelu_apprx_tanh,
                            bias=b1t_sb[:, e * FC + fc : e * FC + fc + 1],
                            scale=1.0 / WSCALE)

                for t in range(QT):
                    pya = ps_mm.tile([128, 512], F32, tag="mm", name="ps_ya")
                    pyb = ps_cx.tile([128, 512], F32, tag="cx", name="ps_yb")
                    for fp in range(FC // 2):
                        nc.tensor.matmul(
                            pya[:, :384],
                            hidT[fp][:, :, t * 128 : (t + 1) * 128],
                            w2_sb[fp][:, :, 0:384],
                            start=(fp == 0), stop=(fp == FC // 2 - 1),
                            perf_mode=mybir.MatmulPerfMode.DoubleRow)
                        nc.tensor.matmul(
                            pyb[:, :384],
                            hidT[fp][:, :, t * 128 : (t + 1) * 128],
                            w2_sb[fp][:, :, 384:768],
                            start=(fp == 0), stop=(fp == FC // 2 - 1),
                            perf_mode=mybir.MatmulPerfMode.DoubleRow)
                    for hf, py in ((0, pya), (1, pyb)):
                        ys = junkp.tile([128, 384], F32, tag="ys", bufs=2,
                                        name="ys")
                        nc.scalar.activation(ys[:], py[:, :384], AF.Identity,
                                             bias=0.0,
                                             scale=gates32[t][:, e : e + 1])
                        nc.vector.tensor_add(
                            xres[t][:, hf * 384 : (hf + 1) * 384],
                            xres[t][:, hf * 384 : (hf + 1) * 384], ys[:])
                    if e == E - 1:
                        nc.sync.dma_start(out_d[t * 128 : (t + 1) * 128, :],
                                          xres[t][:])
            gtp_cm.__exit__(None, None, None)

    nc.compile()
    return nc


def prepare_inputs(inputs):
    x = np.asarray(inputs["x"], np.float32)
    ln1_g = np.asarray(inputs["ln1_g"], np.float32)
    ln1_b = np.asarray(inputs["ln1_b"], np.float32)
    Wq = np.asarray(inputs["Wq"], np.float32)
    Wk = np.asarray(inputs["Wk"], np.float32)
    Wv = np.asarray(inputs["Wv"], np.float32)
    Wo = np.asarray(inputs["Wo"], np.float32)
    Wt = np.asarray(inputs["Wt"], np.float32)
    bt = np.asarray(inputs["bt"], np.float32)
    ln2_g = np.asarray(inputs["ln2_g"], np.float32)
    ln2_b = np.asarray(inputs["ln2_b"], np.float32)
    Wg = np.asarray(inputs["Wg"], np.float32)
    W1 = np.asarray(inputs["W1"], np.float32)
    b1 = np.asarray(inputs["b1"], np.float32)
    W2 = np.asarray(inputs["W2"], np.float32)
    b2 = np.asarray(inputs["b2"], np.float32)

    bv = ln1_b @ Wv
    if np.any(b2) or np.any(bv):
        raise NotImplementedError("nonzero b2 / ln1_b@Wv path not implemented")

    wq = (ln1_g[:, None] * Wq).astype(BF16NP)
    wk = (ln1_g[:, None] * Wk).astype(BF16NP)
    wv = (ln1_g[:, None] * Wv).astype(BF16NP)
    wo = Wo.astype(BF16NP)
    bq = ln1_b @ Wq
    bk = ln1_b @ Wk
    bqk = np.concatenate([bq.reshape(DC, 128).T, bk.reshape(DC, 128).T],
                         axis=1).astype(np.float32)
    has_qkbias = bool(np.any(bqk))

    wg = (ln2_g[:, None] * Wg).astype(BF16NP)
    wg_r = np.ascontiguousarray(
        wg.reshape(DC, 128, E).transpose(1, 0, 2).reshape(128, DC * E))
    bg = (ln2_b @ Wg).reshape(1, E).astype(np.float32)
    wg32 = (ln2_g[:, None] * Wg).astype(np.float32)
    wg_bcast = np.ascontiguousarray(np.broadcast_to(
        wg32.T.reshape(1, E * D), (128, E * D)))
    swg = wg32.sum(axis=0)

    # fp8 DoubleRow layout for W1: [e, fcg, p, (c i fgrp)] where the
    # contraction index is d = c*256 + i*128 + p.
    w1 = ((ln2_g[None, :, None] * W1) * WSCALE).astype(FP8NP)
    w1r = np.ascontiguousarray(
        w1.reshape(E, 3, 2, 128, DC, 512).transpose(0, 4, 3, 1, 2, 5)
        .reshape(E, DC, 128, 3 * 2 * 512))
    b1_tot = (b1 + np.einsum("d,edf->ef", ln2_b, W1)).astype(np.float32)
    b1t = np.ascontiguousarray(
        b1_tot.reshape(E, FC, 128).transpose(2, 0, 1).reshape(128, E * FC))
    # fp8 DoubleRow layout for W2: [e, fp, p, (i d)], contraction index
    # f = fp*256 + i*128 + p.
    w2 = (W2 * WSCALE).astype(FP8NP)
    w2r = np.ascontiguousarray(
        w2.reshape(E, FC // 2, 2, 128, D).transpose(0, 1, 3, 2, 4)
        .reshape(E, FC // 2, 128, 2 * D))
    wt_t = np.ascontiguousarray(Wt.T).astype(BF16NP).reshape(1, 2 * D)

    meta = {"bt0": float(bt[0]), "bt1": float(bt[1]),
            "has_qkbias": has_qkbias, "swg": [float(v) for v in swg]}
    shared = {
        "wq": wq, "wk": wk, "wv": wv, "wo": wo, "wt_t": wt_t,
        "wg_r": wg_r, "bg": bg, "w1r": w1r, "w2r": w2r, "b1t": b1t,
        "bqk": bqk, "wg_bcast": wg_bcast,
    }
    in_maps = []
    for i in range(N_CORES):
        b, half = i // 2, i % 2
        m = dict(shared)
        m["x_kv"] = np.ascontiguousarray(x[b])
        m["x_q"] = np.ascontiguousarray(x[b, half * NQ : (half + 1) * NQ])
        in_maps.append(m)
    return meta, in_maps


def kernel(**inputs):
    meta, in_maps = prepare_inputs(inputs)
    key = ("v1", meta["has_qkbias"], meta["bt0"], meta["bt1"], tuple(meta["swg"]))
    if key not in _CACHED:
        _CACHED[key] = build_program(meta)
    nc = _CACHED[key]

    res = run_bass_kernel_spmd(nc, in_maps, list(range(N_CORES)),
                               trace=bool(inputs.get("_trace", False)))
    out = np.empty((B, S, D), np.float32)
    for i in range(N_CORES):
        b, half = i // 2, i % 2
        out[b, half * NQ : (half + 1) * NQ] = res.results[i]["out"]
    if inputs.get("_want_time", False):
        return out, res
    return out
